# revision 1
# baseline (speedup 1.0000x reference)
"""Trainium2 Bass kernel: frequency-domain regularized (Wiener) deconvolution.

Reference computation (B=16, T=8192, C=8, FIL=16):
    h  = fft(w_real + i*w_imag)            # (FIL, T)
    g  = conj(h) / (|h|^2 + s)             # (FIL, T)
    xf = fft(x, axis=T)                    # per (b, c) row
    y  = real(ifft(xf[:,None,:,:] * g[None,:,None,:]))
    out = y -> (B, T, FIL*C) + bias

Sharding: data-parallel over batch across 8 cores (2 batches/core); filter
params replicated.  FFTs are 4-step Cooley-Tukey matmuls on the PE array
(T = N2*N1, N2=128, N1=64; n = n1 + N1*n2, k = k2 + N2*k1):

  forward:  M1 (contract n2, fp32r) -> twiddle W^(n1 k2) (DVE, broadcast AP)
            -> PE transpose T1 -> M2 (contract n1, stacked-complex K)
            -> Z0 [k1r;k1i | (row,k2)]
  filter:   G = conj(H)/(|H|^2+s) computed on-device from w/s via the same
            forward machinery; assembled into stacked tiles [Gr;Gr], [-Gi;Gi]
  inverse:  3-op complex multiply by G (stacked-swap trick) -> M3 (contract
            k1) -> downcast bf16 -> PE transpose T2 -> M4 per-n1' with the
            inverse twiddle folded into static bf16 weights; bias added on
            PSUM evacuation; direct strided DMA to the output layout.
"""
import sys

sys.path.insert(0, "/opt/trn_rl_repo")

import numpy as np


def _get_cc():
    import concourse.bacc as bacc
    import concourse.mybir as mybir
    import concourse.tile as tile
    return bacc, mybir, tile


class Cfg:
    def __init__(self, T=8192, N2=128, N1=64, BL=2, C=8, FIL=16):
        assert N1 * N2 == T
        self.T, self.N2, self.N1, self.BL, self.C, self.FIL = T, N2, N1, BL, C, FIL
        self.ROWS = BL * C
        self.FC = FIL * C


FULL = Cfg()


def host_consts(cfg):
    """Static (input-independent) weights, as fp32 numpy arrays."""
    T, N1, N2 = cfg.T, cfg.N1, cfg.N2
    f32 = np.float32
    cs = {}
    a2 = np.arange(N2)
    a1 = np.arange(N1)
    F2 = np.exp(-2j * np.pi * np.outer(a2, a2) / N2)        # [n2, k2]
    cs["c_F2r"] = F2.real.astype(f32)
    cs["c_F2i"] = F2.imag.astype(f32)
    cs["c_F2in"] = (-F2.imag).astype(f32)
    Tw = np.exp(-2j * np.pi * np.outer(a2, a1) / T)         # [k2, n1]
    cs["c_Twr"] = Tw.real.astype(f32)
    cs["c_Twi"] = Tw.imag.astype(f32)
    cs["c_Twin"] = (-Tw.imag).astype(f32)
    F1 = np.exp(-2j * np.pi * np.outer(a1, a1) / N1)        # [n1, k1]
    cs["c_M2"] = np.hstack([np.vstack([F1.real, -F1.imag]),
                            np.vstack([F1.imag, F1.real])]).astype(f32)
    Fb1 = np.exp(2j * np.pi * np.outer(a1, a1) / N1)        # [k1, n1']
    cs["c_M3"] = np.hstack([np.vstack([Fb1.real, -Fb1.imag]),
                            np.vstack([Fb1.imag, Fb1.real])]).astype(f32)
    # M4 per-n1' weights, inverse twiddle folded in:
    #   L_{n1'}[k2, n2'] = exp(+2j pi k2 n2'/N2) * exp(+2j pi n1' k2 / T) / T
    Fb2 = np.exp(2j * np.pi * np.outer(a2, a2) / N2)        # [k2, n2']
    ph = np.exp(2j * np.pi * np.outer(a1, a2) / T)          # [n1', k2]
    L = Fb2[None, :, :] * ph[:, :, None] / T                # [n1', k2, n2']
    Lr = L.real.transpose(1, 0, 2).reshape(N2, N1 * N2)     # [k2, (n1', n2')]
    Lin = (-L.imag).transpose(1, 0, 2).reshape(N2, N1 * N2)
    cs["c_L"] = np.concatenate([Lr, Lin], axis=1).astype(f32)  # [k2 | (ri, n1', n2')]
    cs["c_idr"] = np.eye(N2, dtype=f32)
    cs["c_ones"] = np.ones((1, 1), dtype=f32)  # resized at input time
    cs["c_idb"] = np.eye(2 * N1, dtype=f32)
    return cs


def build_nc(cfg, debug_dumps=False):
    bacc, mybir, tile = _get_cc()
    F32, F32R, BF16 = mybir.dt.float32, mybir.dt.float32r, mybir.dt.bfloat16
    AL = mybir.AluOpType
    T, N1, N2, BL, C, FIL = cfg.T, cfg.N1, cfg.N2, cfg.BL, cfg.C, cfg.FIL
    ROWS, FC = cfg.ROWS, cfg.FC
    N1s = 2 * N1          # stacked (real; imag) partition dim
    KF = FIL * N2         # H/G free size, (f, k2) order
    RN = ROWS * N2        # Z0 free size, (row, k2) order
    KB = C * N2           # per-(b,f) inverse free size, (c, k2) order
    MCH = 512             # matmul free-dim chunk (one PSUM bank of fp32)

    nc = bacc.Bacc("TRN2", debug=False)

    xs_d = nc.dram_tensor("xs", [BL, T, C], F32R, kind="ExternalInput")
    wr_d = nc.dram_tensor("wr", [FIL, T], F32R, kind="ExternalInput")
    wi_d = nc.dram_tensor("wi", [FIL, T], F32R, kind="ExternalInput")
    srep_d = nc.dram_tensor("srep", [N1, KF], F32, kind="ExternalInput")
    brep_d = nc.dram_tensor("brep", [N2, FC], F32R, kind="ExternalInput")
    cdef = [
        ("c_F2r", [N2, N2], F32R), ("c_F2i", [N2, N2], F32R), ("c_F2in", [N2, N2], F32R),
        ("c_Twr", [N2, N1], F32), ("c_Twi", [N2, N1], F32), ("c_Twin", [N2, N1], F32),
        ("c_M2", [N1s, N1s], F32R), ("c_M3", [N1s, N1s], BF16),
        ("c_L", [N2, 2 * N1 * N2], BF16),
        ("c_idr", [N2, N2], F32R), ("c_ones", [1, N2], F32R), ("c_idb", [N1s, N1s], BF16),
    ]
    cd = {}
    for name, shape, dt_ in cdef:
        cd[name] = nc.dram_tensor(name, shape, dt_, kind="ExternalInput")
    out_d = nc.dram_tensor("out", [BL, T, FC], F32, kind="ExternalOutput")
    dbg = {}
    if debug_dumps:
        for nm, shape in [("dBT", [N1s, RN]), ("dZ0A", [N1s, RN]), ("dHs", [N1s, KF]),
                          ("dG1", [N1s, KF]), ("dG2", [N1s, KF]), ("dDT0", [N2, 2 * N1 * FC])]:
            dbg[nm] = nc.dram_tensor(nm, shape, F32, kind="ExternalOutput")

    def chunks(total):
        return [(c0, min(total, c0 + MCH)) for c0 in range(0, total, MCH)]

    with tile.TileContext(nc) as tc:
        with tc.tile_pool(name="consts", bufs=1) as cpool, \
             tc.tile_pool(name="spec", bufs=1) as spool, \
             tc.tile_pool(name="gt", bufs=1) as gpool:
            ct = {}
            for name, shape, dt_ in cdef:
                t_ = cpool.tile(shape, dt_, tag=name)
                if name != "c_L":
                    nc.sync.dma_start(out=t_, in_=cd[name].ap())
                ct[name] = t_
            brep = cpool.tile([N2, FC], F32R, tag="brep")
            nc.sync.dma_start(out=brep, in_=brep_d.ap())
            srep = cpool.tile([N1, KF], F32, tag="srep")
            nc.sync.dma_start(out=srep, in_=srep_d.ap())

            Z0A = spool.tile([N1s, RN], BF16, tag="Z0A")   # [k1r;k1i | (row,k2)]
            Z0B = spool.tile([N1s, RN], BF16, tag="Z0B")   # [k1i;k1r | (row,k2)]
            G1 = gpool.tile([N1s, KF], BF16, tag="G1")     # [ Gr;Gr | (f,k2)]
            G2 = gpool.tile([N1s, KF], BF16, tag="G2")     # [-Gi;Gi | (f,k2)]
            from contextlib import ExitStack
            _fes = ExitStack()
            fwdbig = _fes.enter_context(tc.tile_pool(name="fwdbig", bufs=1))
            BT = fwdbig.tile([N1s, RN], F32R, tag="BT")    # [n1r;n1i | (row,k2)]
            BTH = fwdbig.tile([N1s, KF], F32R, tag="BTH")
            Hs = fwdbig.tile([N1s, KF], F32, tag="Hs")

            # ================= forward FFT of x rows =================
            with tc.tile_pool(name="fx", bufs=1) as fp, \
                 tc.tile_pool(name="fxp", bufs=1, space="PSUM") as fps, \
                 tc.tile_pool(name="t1p", bufs=2, space="PSUM") as t1ps:
                for b in range(BL):
                    xt = fp.tile([N2, N1 * C], F32R, tag=f"xt{b}")
                    nc.sync.dma_start(
                        out=xt, in_=xs_d.ap()[b].rearrange("(p q) c -> p (q c)", p=N2))
                    ps = fps.tile([N2, 2 * N1 * C], F32, tag=f"Aps{b}")
                    for comp, w in ((0, "c_F2r"), (1, "c_F2i")):
                        for c0, c1 in chunks(N1 * C):
                            nc.tensor.matmul(
                                ps[:, comp * N1 * C + c0: comp * N1 * C + c1],
                                ct[w], xt[:, c0:c1], start=True, stop=True)
                    # twiddle: Bq = A * W^(n1 k2); A free = (n1, c)
                    Ar = ps[:, :N1 * C].rearrange("p (n c) -> p n c", c=C)
                    Ai = ps[:, N1 * C:].rearrange("p (n c) -> p n c", c=C)
                    Bc = fp.tile([N2, 2 * N1 * C], F32R, tag=f"Bc{b}")
                    u = fp.tile([N2, N1 * C], F32, tag=f"u{b}")
                    v = fp.tile([N2, N1 * C], F32, tag=f"v{b}")

                    def bcx(w):
                        return ct[w][:, :, None].broadcast_to([N2, N1, C])

                    uv = u.rearrange("p (n c) -> p n c", c=C)
                    vv = v.rearrange("p (n c) -> p n c", c=C)
                    Brv = Bc[:, :N1 * C].rearrange("p (n c) -> p n c", c=C)
                    Biv = Bc[:, N1 * C:].rearrange("p (n c) -> p n c", c=C)
                    u2 = fp.tile([N2, N1 * C], F32, tag=f"u2{b}")
                    v2_ = fp.tile([N2, N1 * C], F32, tag=f"v2{b}")
                    u2v = u2.rearrange("p (n c) -> p n c", c=C)
                    v2v = v2_.rearrange("p (n c) -> p n c", c=C)
                    # gpsimd cannot read PSUM: stage A into SBUF via ACT for its half
                    Asb = fp.tile([N2, 2 * N1 * C], F32, tag=f"Asb{b}")
                    nc.scalar.copy(out=Asb, in_=ps)
                    Asr = Asb[:, :N1 * C].rearrange("p (n c) -> p n c", c=C)
                    Asi = Asb[:, N1 * C:].rearrange("p (n c) -> p n c", c=C)
                    nc.vector.tensor_tensor(out=uv, in0=Ar, in1=bcx("c_Twr"), op=AL.mult)
                    nc.vector.tensor_tensor(out=vv, in0=Ai, in1=bcx("c_Twin"), op=AL.mult)
                    nc.vector.tensor_tensor(out=Brv, in0=uv, in1=vv, op=AL.add)
                    nc.gpsimd.tensor_tensor(out=u2v, in0=Asr, in1=bcx("c_Twi"), op=AL.mult)
                    nc.gpsimd.tensor_tensor(out=v2v, in0=Asi, in1=bcx("c_Twr"), op=AL.mult)
                    nc.gpsimd.tensor_tensor(out=Biv, in0=u2v, in1=v2v, op=AL.add)
                    # T1: one fused transpose per row: [N2 | (comp,n1)] -> [(comp,n1) | N2]
                    Bview = Bc.rearrange("p (m n c) -> p m n c", m=2, c=C)
                    for c in range(C):
                        tp = t1ps.tile([N1s, N2], F32R, tag="t1")
                        nc.tensor.transpose(tp, Bview[:, :, :, c], ct["c_idr"])
                        row = b * C + c
                        nc.scalar.copy(out=BT[:, row * N2:(row + 1) * N2], in_=tp)

            # M2: Z0 = F1-stack^T @ BT
            with tc.tile_pool(name="m2p", bufs=1, space="PSUM") as m2ps:
                ps = m2ps.tile([N1s, RN], F32, tag="m2")
                for c0, c1 in chunks(RN):
                    nc.tensor.matmul(ps[:, c0:c1], ct["c_M2"], BT[:, c0:c1],
                                     start=True, stop=True)
                nc.vector.tensor_copy(out=Z0A, in_=ps)
            nc.sync.dma_start(out=Z0B[N1:, :], in_=Z0A[:N1, :])
            nc.sync.dma_start(out=Z0B[:N1, :], in_=Z0A[N1:, :])

            # ================= forward FFT of w rows (H), then G =================
            with tc.tile_pool(name="fh", bufs=1) as hp, \
                 tc.tile_pool(name="fhp", bufs=1, space="PSUM") as hps, \
                 tc.tile_pool(name="t1hp", bufs=2, space="PSUM") as t1hps:
                wtr = hp.tile([N2, FIL * N1], F32R, tag="wtr")
                wti = hp.tile([N2, FIL * N1], F32R, tag="wti")
                nc.sync.dma_start(out=wtr.rearrange("p (f n) -> p f n", f=FIL),
                                  in_=wr_d.ap().rearrange("f (p n) -> p f n", p=N2))
                nc.sync.dma_start(out=wti.rearrange("p (f n) -> p f n", f=FIL),
                                  in_=wi_d.ap().rearrange("f (p n) -> p f n", p=N2))
                ps = hps.tile([N2, 2 * FIL * N1], F32, tag="Hps")
                for c0, c1 in chunks(FIL * N1):
                    nc.tensor.matmul(ps[:, c0:c1], ct["c_F2r"], wtr[:, c0:c1],
                                     start=True, stop=False)
                    nc.tensor.matmul(ps[:, c0:c1], ct["c_F2in"], wti[:, c0:c1],
                                     start=False, stop=True)
                    d0 = FIL * N1
                    nc.tensor.matmul(ps[:, d0 + c0:d0 + c1], ct["c_F2i"], wtr[:, c0:c1],
                                     start=True, stop=False)
                    nc.tensor.matmul(ps[:, d0 + c0:d0 + c1], ct["c_F2r"], wti[:, c0:c1],
                                     start=False, stop=True)
                # twiddle; free = (f, n1), broadcast over f (outer)
                Ar = ps[:, :FIL * N1].rearrange("p (f n) -> p f n", f=FIL)
                Ai = ps[:, FIL * N1:].rearrange("p (f n) -> p f n", f=FIL)
                BHc = hp.tile([N2, FIL * 2 * N1], F32R, tag="BHc")
                u = hp.tile([N2, FIL * N1], F32, tag="uh")
                v = hp.tile([N2, FIL * N1], F32, tag="vh")

                def bch(w):
                    return ct[w][:, None, :].broadcast_to([N2, FIL, N1])

                uv = u.rearrange("p (f n) -> p f n", f=FIL)
                vv = v.rearrange("p (f n) -> p f n", f=FIL)
                BHv = BHc.rearrange("p (f m n) -> p f m n", f=FIL, m=2)
                nc.vector.tensor_tensor(out=uv, in0=Ar, in1=bch("c_Twr"), op=AL.mult)
                nc.vector.tensor_tensor(out=vv, in0=Ai, in1=bch("c_Twin"), op=AL.mult)
                nc.vector.tensor_tensor(out=BHv[:, :, 0, :], in0=uv, in1=vv, op=AL.add)
                nc.vector.tensor_tensor(out=uv, in0=Ar, in1=bch("c_Twi"), op=AL.mult)
                nc.vector.tensor_tensor(out=vv, in0=Ai, in1=bch("c_Twr"), op=AL.mult)
                nc.vector.tensor_tensor(out=BHv[:, :, 1, :], in0=uv, in1=vv, op=AL.add)
                for f in range(FIL):
                    tp = t1hps.tile([N1s, N2], F32R, tag="t1h")
                    nc.tensor.transpose(tp, BHc[:, f * 2 * N1:(f + 1) * 2 * N1], ct["c_idr"])
                    nc.scalar.copy(out=BTH[:, f * N2:(f + 1) * N2], in_=tp)

            with tc.tile_pool(name="m2hp", bufs=1, space="PSUM") as m2hps:
                ps = m2hps.tile([N1s, KF], F32, tag="m2h")
                for c0, c1 in chunks(KF):
                    nc.tensor.matmul(ps[:, c0:c1], ct["c_M2"], BTH[:, c0:c1],
                                     start=True, stop=True)
                nc.vector.tensor_copy(out=Hs, in_=ps)

            # G = conj(H) / (|H|^2 + s): all DVE ops at partition base 0;
            # cross-partition marshaling via SBUF->SBUF DMA.
            with tc.tile_pool(name="g", bufs=1) as gp:
                sq = gp.tile([N1s, KF], F32, tag="sq")
                nc.scalar.square(sq, Hs)
                sqB = gp.tile([N1, KF], F32, tag="sqB")
                HiB = gp.tile([N1, KF], F32, tag="HiB")
                nc.sync.dma_start(out=sqB, in_=sq[N1:, :])
                nc.sync.dma_start(out=HiB, in_=Hs[N1:, :])
                d = gp.tile([N1, KF], F32, tag="d")
                nc.vector.tensor_tensor(out=d, in0=sq[:N1, :], in1=sqB, op=AL.add)
                nc.vector.tensor_tensor(out=d, in0=d, in1=srep, op=AL.add)
                r = gp.tile([N1, KF], F32, tag="r")
                nc.vector.reciprocal(out=r, in_=d)
                rn = gp.tile([N1, KF], F32, tag="rn")
                nc.vector.tensor_scalar_mul(out=rn, in0=r, scalar1=-1.0)
                gtmp = gp.tile([N1, KF], BF16, tag="gtmp")
                nc.vector.tensor_tensor(out=G1[:N1, :], in0=Hs[:N1, :], in1=r, op=AL.mult)
                nc.vector.tensor_tensor(out=G2[:N1, :], in0=HiB, in1=r, op=AL.mult)
                nc.vector.tensor_tensor(out=gtmp, in0=HiB, in1=rn, op=AL.mult)
                nc.sync.dma_start(out=G1[N1:, :], in_=G1[:N1, :])
                nc.sync.dma_start(out=G2[N1:, :], in_=gtmp)

            if debug_dumps:
                F32b = F32
                nc.sync.dma_start(out=dbg["dBT"].ap(), in_=BT.bitcast(F32b))
                nc.gpsimd.dma_start(out=dbg["dZ0A"].ap(), in_=Z0A)
                nc.sync.dma_start(out=dbg["dHs"].ap(), in_=Hs)
                nc.gpsimd.dma_start(out=dbg["dG1"].ap(), in_=G1)
                nc.gpsimd.dma_start(out=dbg["dG2"].ap(), in_=G2)
            _fes.close()
            nc.sync.dma_start(out=ct["c_L"], in_=cd["c_L"].ap())
            # ================= inverse per (b, f) =================
            with tc.tile_pool(name="inv", bufs=2) as ip, \
                 tc.tile_pool(name="invs", bufs=2) as ip1, \
                 tc.tile_pool(name="dt", bufs=2) as dtp, \
                 tc.tile_pool(name="invp", bufs=2, space="PSUM") as ips, \
                 tc.tile_pool(name="t2p", bufs=2, space="PSUM") as t2ps, \
                 tc.tile_pool(name="yp", bufs=2, space="PSUM") as yps, \
                 tc.tile_pool(name="yev", bufs=3) as yp:
                for b in range(BL):
                    DT = dtp.tile([N2, 2 * N1 * FC], BF16, tag="DT")
                    # free layout: (fc, ri, n1) — DMA-transpose dest contiguous per (f,c)
                    dtm = DT.rearrange("p (fc ri n1) -> p ri n1 fc", fc=FC, ri=2, n1=N1)
                    for f in range(FIL):
                        zA = Z0A[:, b * KB:(b + 1) * KB].rearrange("p (c k) -> p c k", c=C)
                        zB = Z0B[:, b * KB:(b + 1) * KB].rearrange("p (c k) -> p c k", c=C)
                        g1 = G1[:, f * N2:(f + 1) * N2][:, None, :].broadcast_to([N1s, C, N2])
                        g2 = G2[:, f * N2:(f + 1) * N2][:, None, :].broadcast_to([N1s, C, N2])
                        veng = nc.gpsimd if (f % 8 == 2) else nc.vector
                        sfx = "g" if f % 3 == 2 else ""
                        zt1 = ip1.tile([N1s, KB], BF16, tag="zt1" + sfx)
                        zt2 = ip1.tile([N1s, KB], BF16, tag="zt2" + sfx)
                        zf = ip.tile([N1s, KB], BF16, tag="zf")
                        z1v = zt1.rearrange("p (c k) -> p c k", c=C)
                        z2v = zt2.rearrange("p (c k) -> p c k", c=C)
                        veng.tensor_tensor(out=z1v, in0=zA, in1=g1, op=AL.mult)
                        veng.tensor_tensor(out=z2v, in0=zB, in1=g2, op=AL.mult)
                        veng.tensor_tensor(out=zf, in0=zt1, in1=zt2, op=AL.add)
                        cps = ips.tile([N1s, KB], F32, tag="cps")
                        for c0, c1 in chunks(KB):
                            nc.tensor.matmul(cps[:, c0:c1], ct["c_M3"], zf[:, c0:c1],
                                             start=True, stop=True)
                        cs_ = ip.tile([N1s, KB], BF16, tag="cs")
                        nc.scalar.copy(out=cs_, in_=cps)  # ACT
                        # T2: PE transposes (bf16, 1cyc/row), one contiguous evac
                        tp = t2ps.tile([N2, C * N1s], BF16, tag="t2")
                        for c in range(C):
                            nc.tensor.transpose(
                                tp[:, c * N1s:(c + 1) * N1s],
                                cs_[:, c * N2:(c + 1) * N2], ct["c_idb"])
                        nc.scalar.copy(
                            out=DT[:, f * C * N1s:(f + 1) * C * N1s], in_=tp)
                    if debug_dumps and b == 0:
                        nc.gpsimd.dma_start(out=dbg["dDT0"].ap(), in_=DT)
                    # M4, batched 4 n1' per PSUM bank
                    NB = max(1, min(N1, MCH // FC))
                    for g0 in range(0, N1, NB):
                        gn = min(NB, N1 - g0)
                        ypsum = yps.tile([N2, NB * FC], F32, tag="yps")
                        for j in range(gn):
                            n1p = g0 + j
                            lr = ct["c_L"][:, n1p * N2:(n1p + 1) * N2]
                            li = ct["c_L"][:, (N1 + n1p) * N2:(N1 + n1p + 1) * N2]
                            rr = dtm[:, 0, n1p, :]
                            ri_ = dtm[:, 1, n1p, :]
                            sl = ypsum[:, j * FC:(j + 1) * FC]
                            nc.tensor.matmul(sl, lr, rr, start=(j == 0), stop=False)
                            nc.tensor.matmul(sl, li, ri_, start=False,
                                             stop=(j == gn - 1))
                        yt = yp.tile([N2, NB * FC], F32, tag="yt")
                        bb = brep[:, None, :].broadcast_to([N2, gn, FC])
                        nc.vector.tensor_tensor(
                            out=yt[:, :gn * FC].rearrange("p (j fc) -> p j fc", j=gn),
                            in0=ypsum[:, :gn * FC].rearrange("p (j fc) -> p j fc", j=gn),
                            in1=bb, op=AL.add)
                        nc.sync.dma_start(
                            out=out_d.ap()[b].rearrange("(n2 n1) fc -> n2 n1 fc", n1=N1)[:, g0:g0 + gn, :],
                            in_=yt[:, :gn * FC].rearrange("p (j fc) -> p j fc", j=gn))

    nc.compile()
    return nc


def host_inputs(cfg, x_sh, w_real, w_imag, s, b):
    """Build the per-core in_map (numpy) for one core's batch shard."""
    import ml_dtypes
    cs = host_consts(cfg)
    N1, N2, FIL, C, FC = cfg.N1, cfg.N2, cfg.FIL, cfg.C, cfg.FC
    m = {
        "xs": np.ascontiguousarray(x_sh, dtype=np.float32),
        "wr": np.ascontiguousarray(w_real, dtype=np.float32),
        "wi": np.ascontiguousarray(w_imag, dtype=np.float32),
        "srep": np.broadcast_to(s.reshape(1, FIL, 1), (N1, FIL, N2)).reshape(N1, FIL * N2).astype(np.float32).copy(),
        "brep": np.broadcast_to(b.reshape(1, FC), (N2, FC)).astype(np.float32).copy(),
    }
    cs["c_ones"] = np.ones((1, N2), dtype=np.float32)
    for k, v in cs.items():
        if k in ("c_L", "c_M3", "c_idb"):
            m[k] = v.astype(ml_dtypes.bfloat16)
        else:
            m[k] = v
    return m


_NC_CACHE = {}


def kernel(x, w_real, w_imag, s, b):
    """Full-input entry point: shard over 8 cores, run, gather."""
    from concourse.bass_utils import run_bass_kernel_spmd
    cfg = FULL
    n_cores = 8
    key = "full"
    if key not in _NC_CACHE:
        _NC_CACHE[key] = build_nc(cfg)
    nc = _NC_CACHE[key]
    x = np.asarray(x, dtype=np.float32)
    w_real = np.asarray(w_real, dtype=np.float32)
    w_imag = np.asarray(w_imag, dtype=np.float32)
    s = np.asarray(s, dtype=np.float32)
    b = np.asarray(b, dtype=np.float32)
    in_maps = []
    for i in range(n_cores):
        x_sh = x[i * cfg.BL:(i + 1) * cfg.BL]
        in_maps.append(host_inputs(cfg, x_sh, w_real, w_imag, s, b))
    res = run_bass_kernel_spmd(nc, in_maps, core_ids=list(range(n_cores)))
    outs = [res.results[i]["out"] for i in range(n_cores)]
    return np.concatenate(outs, axis=0).astype(np.float32)



# revision 14
# speedup vs baseline: 1.1874x; 1.1874x over previous
"""Trainium2 Bass kernel: frequency-domain regularized (Wiener) deconvolution.

Reference computation (B=16, T=8192, C=8, FIL=16):
    h  = fft(w_real + i*w_imag)            # (FIL, T)
    g  = conj(h) / (|h|^2 + s)             # (FIL, T)
    xf = fft(x, axis=T)                    # per (b, c) row
    y  = real(ifft(xf[:,None,:,:] * g[None,:,None,:]))
    out = y -> (B, T, FIL*C) + bias

Sharding: data-parallel over batch across 8 cores (2 batches/core); filter
params replicated.  FFTs are 4-step Cooley-Tukey matmuls on the PE array
(T = N2*N1, N2=128, N1=64; n = n1 + N1*n2, k = k2 + N2*k1):

  forward:  M1 (contract n2, fp32r) -> twiddle W^(n1 k2) (DVE+Pool)
            -> PE transpose T1 (batched PSUM banks, ACT evac) -> M2
            (contract n1, stacked-complex K) -> Z0 [k1r;k1i | (row,k2)]
  filter:   G = conj(H)/(|H|^2+s) on-device; elementwise pipeline runs in a
            partition-packed [128,1024] layout (Pool+DVE+ACT), assembled into
            stacked bf16 tiles G1=[Gr;Gr], G2=[-Gi;Gi] via SBUF-SBUF DMAs
  inverse:  per f: two DVE mults zt1=Z0A*G1f, zt2=Z0B*G2f (bias folded into
            the k=0 bin of zt1); stage-1 iFFT contracts k1 with the DATA as
            matmul weights (out partitions = k2) and the complex add FUSED
            into PSUM accumulation -> no PE transpose, no extra DVE add;
            ACT/DVE evacuate [k2 | (b,ri,n1',f,c)] bf16; M4 contracts k2
            per (b,n1') with inverse twiddle folded into static bf16 weights;
            fp32 PSUM -> SBUF -> DMA straight to the output layout.
"""
import sys

sys.path.insert(0, "/opt/trn_rl_repo")

import numpy as np


def _get_cc():
    import concourse.bacc as bacc
    import concourse.mybir as mybir
    import concourse.tile as tile
    return bacc, mybir, tile


class Cfg:
    def __init__(self, T=8192, N2=128, N1=64, BL=2, C=8, FIL=16):
        assert N1 * N2 == T
        self.T, self.N2, self.N1, self.BL, self.C, self.FIL = T, N2, N1, BL, C, FIL
        self.ROWS = BL * C
        self.FC = FIL * C


FULL = Cfg()


def host_consts(cfg):
    """Static (input-independent) weights, as fp32 numpy arrays."""
    T, N1, N2 = cfg.T, cfg.N1, cfg.N2
    f32 = np.float32
    cs = {}
    a2 = np.arange(N2)
    a1 = np.arange(N1)
    F2 = np.exp(-2j * np.pi * np.outer(a2, a2) / N2)        # [n2, k2]
    cs["c_F2r"] = F2.real.astype(f32)
    cs["c_F2i"] = F2.imag.astype(f32)
    cs["c_F2in"] = (-F2.imag).astype(f32)
    Tw = np.exp(-2j * np.pi * np.outer(a2, a1) / T)         # [k2, n1]
    cs["c_Twr"] = Tw.real.astype(f32)
    cs["c_Twi"] = Tw.imag.astype(f32)
    cs["c_Twin"] = (-Tw.imag).astype(f32)
    F1 = np.exp(-2j * np.pi * np.outer(a1, a1) / N1)        # [n1, k1]
    cs["c_M2"] = np.hstack([np.vstack([F1.real, -F1.imag]),
                            np.vstack([F1.imag, F1.real])]).astype(f32)
    Fb1 = np.exp(2j * np.pi * np.outer(a1, a1) / N1)        # [k1, n1']
    cs["c_M3"] = np.hstack([np.vstack([Fb1.real, -Fb1.imag]),
                            np.vstack([Fb1.imag, Fb1.real])]).astype(f32)
    # M4 per-n1' weights, inverse twiddle folded in:
    #   L_{n1'}[k2, n2'] = exp(+2j pi k2 n2'/N2) * exp(+2j pi n1' k2 / T) / T
    Fb2 = np.exp(2j * np.pi * np.outer(a2, a2) / N2)        # [k2, n2']
    ph = np.exp(2j * np.pi * np.outer(a1, a2) / T)          # [n1', k2]
    L = Fb2[None, :, :] * ph[:, :, None] / T                # [n1', k2, n2']
    Lr = L.real.transpose(1, 0, 2).reshape(N2, N1 * N2)     # [k2, (n1', n2')]
    Lin = (-L.imag).transpose(1, 0, 2).reshape(N2, N1 * N2)
    cs["c_L"] = np.concatenate([Lr, Lin], axis=1).astype(f32)  # [k2 | (ri, n1', n2')]
    cs["c_idr"] = np.eye(N2, dtype=f32)
    return cs


def build_nc(cfg, debug_dumps=False):
    bacc, mybir, tile = _get_cc()
    F32, F32R, BF16 = mybir.dt.float32, mybir.dt.float32r, mybir.dt.bfloat16
    AL = mybir.AluOpType
    T, N1, N2, BL, C, FIL = cfg.T, cfg.N1, cfg.N2, cfg.BL, cfg.C, cfg.FIL
    ROWS, FC = cfg.ROWS, cfg.FC
    N1s = 2 * N1          # stacked (real; imag) partition dim = 128
    KF = FIL * N2         # H/G free size, (f, k2) order = 2048
    KH = KF // 2          # packed-G free size = 1024
    RN = ROWS * N2        # Z0 free size, (b, c, k2) order = 2048
    MCH = 512             # matmul free-dim chunk (one PSUM bank of fp32)

    nc = bacc.Bacc("TRN2", debug=False)

    xs_d = nc.dram_tensor("xs", [BL, T, C], F32R, kind="ExternalInput")
    wr_d = nc.dram_tensor("wr", [FIL, T], F32R, kind="ExternalInput")
    wi_d = nc.dram_tensor("wi", [FIL, T], F32R, kind="ExternalInput")
    srepP_d = nc.dram_tensor("srepP", [N2, KH], F32, kind="ExternalInput")
    bk0_d = nc.dram_tensor("bk0", [1, FIL * BL * C], F32, kind="ExternalInput")
    cdef = [
        ("c_F2r", [N2, N2], F32R), ("c_F2i", [N2, N2], F32R), ("c_F2in", [N2, N2], F32R),
        ("c_Twr", [N2, N1], F32), ("c_Twi", [N2, N1], F32), ("c_Twin", [N2, N1], F32),
        ("c_M2", [N1s, N1s], F32R), ("c_M3", [N1s, N1s], BF16),
        ("c_L", [N2, 2 * N1 * N2], BF16),
        ("c_idr", [N2, N2], F32R),
    ]
    cd = {}
    for name, shape, dt_ in cdef:
        cd[name] = nc.dram_tensor(name, shape, dt_, kind="ExternalInput")
    out_d = nc.dram_tensor("out", [BL, T, FC], F32, kind="ExternalOutput")
    dbg = {}
    if debug_dumps:
        for nm, shape, ddt in [("dZ0A", [N1s, RN], BF16), ("dZ0B", [N1s, RN], BF16),
                               ("dG1", [N1s, KF], BF16), ("dG2", [N1s, KF], BF16),
                               ("dZT1", [N1s, RN], BF16), ("dZT2", [N1s, RN], BF16),
                               ("dHs", [N1s, KF], F32),
                               ("dDT", [N2, BL * 2 * N1 * FIL * C], BF16)]:
            dbg[nm] = nc.dram_tensor(nm, shape, ddt, kind="ExternalOutput")

    def chunks(total):
        return [(c0, min(total, c0 + MCH)) for c0 in range(0, total, MCH)]

    with tile.TileContext(nc) as tc:
        with tc.tile_pool(name="consts", bufs=1) as cpool, \
             tc.tile_pool(name="spec", bufs=1) as spool, \
             tc.tile_pool(name="gt", bufs=1) as gpool:
            ct = {}
            for name, shape, dt_ in cdef:
                t_ = cpool.tile(shape, dt_, tag=name)
                if name != "c_L":
                    nc.sync.dma_start(out=t_, in_=cd[name].ap())
                ct[name] = t_
            bk0 = cpool.tile([1, FIL * BL * C], F32, tag="bk0")
            nc.sync.dma_start(out=bk0, in_=bk0_d.ap())
            srepP = cpool.tile([N2, KH], F32, tag="srepP")
            nc.sync.dma_start(out=srepP, in_=srepP_d.ap())

            Z0A = spool.tile([N1s, RN], BF16, tag="Z0A")   # [k1r;k1i | (b,c,k2)]
            Z0B = spool.tile([N1s, RN], BF16, tag="Z0B")   # [k1i;k1r | (b,c,k2)]
            G1 = gpool.tile([N1s, KF], BF16, tag="G1")     # [ Gr;Gr | (f,k2)]
            G2 = gpool.tile([N1s, KF], BF16, tag="G2")     # [-Gi;Gi | (f,k2)]

            # ================= forward FFT of w rows (H) =================
            with tc.tile_pool(name="fh", bufs=1) as hp, \
                 tc.tile_pool(name="t1hp", bufs=2, space="PSUM") as t1hps:
                wtr = hp.tile([N2, FIL * N1], F32R, tag="wtr")
                wti = hp.tile([N2, FIL * N1], F32R, tag="wti")
                nc.sync.dma_start(out=wtr.rearrange("p (f n) -> p f n", f=FIL),
                                  in_=wr_d.ap().rearrange("f (p n) -> p f n", p=N2))
                nc.sync.dma_start(out=wti.rearrange("p (f n) -> p f n", f=FIL),
                                  in_=wi_d.ap().rearrange("f (p n) -> p f n", p=N2))
                Hsb = hp.tile([N2, 2 * FIL * N1], F32, tag="Hsb")
                with tc.tile_pool(name="fhp", bufs=1, space="PSUM") as hps:
                    ps = hps.tile([N2, 2 * FIL * N1], F32, tag="Hps")
                    for c0, c1 in chunks(FIL * N1):
                        nc.tensor.matmul(ps[:, c0:c1], ct["c_F2r"], wtr[:, c0:c1],
                                         start=True, stop=False)
                        nc.tensor.matmul(ps[:, c0:c1], ct["c_F2in"], wti[:, c0:c1],
                                         start=False, stop=True)
                        d0 = FIL * N1
                        nc.tensor.matmul(ps[:, d0 + c0:d0 + c1], ct["c_F2i"],
                                         wtr[:, c0:c1], start=True, stop=False)
                        nc.tensor.matmul(ps[:, d0 + c0:d0 + c1], ct["c_F2r"],
                                         wti[:, c0:c1], start=False, stop=True)
                    # stage PSUM -> SBUF (ACT) so the twiddle can run on Pool
                    nc.scalar.copy(out=Hsb, in_=ps)
                # twiddle on Pool; free = (f, n1), broadcast over f (outer)
                Ar = Hsb[:, :FIL * N1].rearrange("p (f n) -> p f n", f=FIL)
                Ai = Hsb[:, FIL * N1:].rearrange("p (f n) -> p f n", f=FIL)
                BHc = hp.tile([N2, FIL * 2 * N1], F32R, tag="BHc")
                u = hp.tile([N2, FIL * N1], F32, tag="uh")
                v = hp.tile([N2, FIL * N1], F32, tag="vh")

                def bch(w):
                    return ct[w][:, None, :].broadcast_to([N2, FIL, N1])

                uv = u.rearrange("p (f n) -> p f n", f=FIL)
                vv = v.rearrange("p (f n) -> p f n", f=FIL)
                BHv = BHc.rearrange("p (f m n) -> p f m n", f=FIL, m=2)
                nc.gpsimd.tensor_tensor(out=uv, in0=Ar, in1=bch("c_Twr"), op=AL.mult)
                nc.gpsimd.tensor_tensor(out=vv, in0=Ai, in1=bch("c_Twin"), op=AL.mult)
                nc.gpsimd.tensor_tensor(out=BHv[:, :, 0, :], in0=uv, in1=vv, op=AL.add)
                nc.gpsimd.tensor_tensor(out=uv, in0=Ar, in1=bch("c_Twi"), op=AL.mult)
                nc.gpsimd.tensor_tensor(out=vv, in0=Ai, in1=bch("c_Twr"), op=AL.mult)
                nc.gpsimd.tensor_tensor(out=BHv[:, :, 1, :], in0=uv, in1=vv, op=AL.add)
                # T1H: PE transposes, 4 per PSUM bank, one ACT evac per bank
                BTH = hp.tile([N1s, KF], F32R, tag="BTH")
                for q in range(FIL // 4):
                    tp = t1hps.tile([N1s, 4 * N2], F32R, tag="t1h")
                    for j in range(4):
                        f = 4 * q + j
                        nc.tensor.transpose(
                            tp[:, j * N2:(j + 1) * N2],
                            BHc[:, f * 2 * N1:(f + 1) * 2 * N1], ct["c_idr"])
                    nc.scalar.copy(out=BTH[:, q * 4 * N2:(q + 1) * 4 * N2], in_=tp)

                with tc.tile_pool(name="m2hp", bufs=1, space="PSUM") as m2hps:
                    ps2 = m2hps.tile([N1s, KF], F32, tag="m2h")
                    for c0, c1 in chunks(KF):
                        nc.tensor.matmul(ps2[:, c0:c1], ct["c_M2"], BTH[:, c0:c1],
                                         start=True, stop=True)
                    Hs = hp.tile([N1s, KF], F32, tag="Hs")
                    nc.scalar.copy(out=Hs, in_=ps2)
                if debug_dumps:
                    nc.sync.dma_start(out=dbg["dHs"].ap(), in_=Hs)

                # ============== G in packed [128, KH] layout ==============
                with tc.tile_pool(name="g", bufs=1) as gp:
                    HrP = gp.tile([N2, KH], F32, tag="HrP")
                    HiP = gp.tile([N2, KH], F32, tag="HiP")
                    nc.sync.dma_start(out=HrP[:N1, :], in_=Hs[:N1, :KH])
                    nc.sync.dma_start(out=HrP[N1:, :], in_=Hs[:N1, KH:])
                    nc.sync.dma_start(out=HiP[:N1, :], in_=Hs[N1:, :KH])
                    nc.sync.dma_start(out=HiP[N1:, :], in_=Hs[N1:, KH:])
                    sq1 = gp.tile([N2, KH], F32, tag="sq1")
                    sq2 = gp.tile([N2, KH], F32, tag="sq2")
                    nc.gpsimd.tensor_tensor(out=sq1, in0=HrP, in1=HrP, op=AL.mult)
                    nc.gpsimd.tensor_tensor(out=sq2, in0=HiP, in1=HiP, op=AL.mult)
                    nc.gpsimd.tensor_tensor(out=sq1, in0=sq1, in1=sq2, op=AL.add)
                    nc.vector.tensor_tensor(out=sq1, in0=sq1, in1=srepP, op=AL.add)
                    r = sq2  # reciprocal result reuses sq2's buffer
                    nc.vector.reciprocal(out=r, in_=sq1)
                    GrPb = gp.tile([N2, KH], BF16, tag="GrPb")
                    GiPb = gp.tile([N2, KH], BF16, tag="GiPb")
                    GiNPb = gp.tile([N2, KH], BF16, tag="GiNPb")
                    nc.vector.tensor_tensor(out=GrPb, in0=HrP, in1=r, op=AL.mult)
                    nc.vector.tensor_tensor(out=GiPb, in0=HiP, in1=r, op=AL.mult)
                    nc.gpsimd.tensor_scalar_mul(out=GiNPb, in0=GiPb, scalar1=-1.0)
                    # unpack packed [128, KH] -> stacked [128, KF]
                    nc.sync.dma_start(out=G1[:N1, :KH], in_=GrPb[:N1, :])
                    nc.sync.dma_start(out=G1[:N1, KH:], in_=GrPb[N1:, :])
                    nc.sync.dma_start(out=G1[N1:, :KH], in_=GrPb[:N1, :])
                    nc.sync.dma_start(out=G1[N1:, KH:], in_=GrPb[N1:, :])
                    # conj(H)*r: Im G = -Hi*r, so GiPb (=Hi*r) is already -Gi
                    nc.sync.dma_start(out=G2[:N1, :KH], in_=GiPb[:N1, :])
                    nc.sync.dma_start(out=G2[:N1, KH:], in_=GiPb[N1:, :])
                    nc.sync.dma_start(out=G2[N1:, :KH], in_=GiNPb[:N1, :])
                    nc.sync.dma_start(out=G2[N1:, KH:], in_=GiNPb[N1:, :])

            # ================= forward FFT of x rows =================
            BT = spool.tile([N1s, RN], F32R, tag="BT")     # [n1r;n1i | (b,c,k2)]
            with tc.tile_pool(name="fx", bufs=1) as fp, \
                 tc.tile_pool(name="fxp", bufs=1, space="PSUM") as fps, \
                 tc.tile_pool(name="t1p", bufs=2, space="PSUM") as t1ps:
                for b in range(BL):
                    xt = fp.tile([N2, N1 * C], F32R, tag=f"xt{b}")
                    nc.sync.dma_start(
                        out=xt, in_=xs_d.ap()[b].rearrange("(p q) c -> p (q c)", p=N2))
                    ps = fps.tile([N2, 2 * N1 * C], F32, tag=f"Aps{b}")
                    for comp, w in ((0, "c_F2r"), (1, "c_F2i")):
                        for c0, c1 in chunks(N1 * C):
                            nc.tensor.matmul(
                                ps[:, comp * N1 * C + c0: comp * N1 * C + c1],
                                ct[w], xt[:, c0:c1], start=True, stop=True)
                    # twiddle: Bq = A * W^(n1 k2); A free = (n1, c)
                    Ar = ps[:, :N1 * C].rearrange("p (n c) -> p n c", c=C)
                    Ai = ps[:, N1 * C:].rearrange("p (n c) -> p n c", c=C)
                    Bc = fp.tile([N2, 2 * N1 * C], F32R, tag=f"Bc{b}")
                    u = fp.tile([N2, N1 * C], F32, tag=f"u{b}")
                    v = fp.tile([N2, N1 * C], F32, tag=f"v{b}")

                    def bcx(w):
                        return ct[w][:, :, None].broadcast_to([N2, N1, C])

                    uv = u.rearrange("p (n c) -> p n c", c=C)
                    vv = v.rearrange("p (n c) -> p n c", c=C)
                    Brv = Bc[:, :N1 * C].rearrange("p (n c) -> p n c", c=C)
                    Biv = Bc[:, N1 * C:].rearrange("p (n c) -> p n c", c=C)
                    u2 = fp.tile([N2, N1 * C], F32, tag=f"u2{b}")
                    v2_ = fp.tile([N2, N1 * C], F32, tag=f"v2{b}")
                    u2v = u2.rearrange("p (n c) -> p n c", c=C)
                    v2v = v2_.rearrange("p (n c) -> p n c", c=C)
                    # gpsimd cannot read PSUM: stage A into SBUF via ACT for its half
                    Asb = fp.tile([N2, 2 * N1 * C], F32, tag=f"Asb{b}")
                    nc.scalar.copy(out=Asb, in_=ps)
                    Asr = Asb[:, :N1 * C].rearrange("p (n c) -> p n c", c=C)
                    Asi = Asb[:, N1 * C:].rearrange("p (n c) -> p n c", c=C)
                    nc.vector.tensor_tensor(out=uv, in0=Ar, in1=bcx("c_Twr"), op=AL.mult)
                    nc.vector.tensor_tensor(out=vv, in0=Ai, in1=bcx("c_Twin"), op=AL.mult)
                    nc.vector.tensor_tensor(out=Brv, in0=uv, in1=vv, op=AL.add)
                    nc.gpsimd.tensor_tensor(out=u2v, in0=Asr, in1=bcx("c_Twi"), op=AL.mult)
                    nc.gpsimd.tensor_tensor(out=v2v, in0=Asi, in1=bcx("c_Twr"), op=AL.mult)
                    nc.gpsimd.tensor_tensor(out=Biv, in0=u2v, in1=v2v, op=AL.add)
                    # T1: PE transposes, 4 per PSUM bank, one ACT evac per bank
                    Bview = Bc.rearrange("p (m n c) -> p m n c", m=2, c=C)
                    for q in range(C // 4):
                        tp = t1ps.tile([N1s, 4 * N2], F32R, tag="t1")
                        for j in range(4):
                            c = 4 * q + j
                            nc.tensor.transpose(tp[:, j * N2:(j + 1) * N2],
                                                Bview[:, :, :, c], ct["c_idr"])
                        row = b * C + 4 * q
                        nc.scalar.copy(out=BT[:, row * N2:(row + 4) * N2], in_=tp)

            # M2: Z0 = F1-stack^T @ BT
            with tc.tile_pool(name="m2p", bufs=1, space="PSUM") as m2ps:
                ps = m2ps.tile([N1s, RN], F32, tag="m2")
                for c0, c1 in chunks(RN):
                    nc.tensor.matmul(ps[:, c0:c1], ct["c_M2"], BT[:, c0:c1],
                                     start=True, stop=True)
                nc.vector.tensor_copy(out=Z0A, in_=ps)
            nc.sync.dma_start(out=Z0B[N1:, :], in_=Z0A[:N1, :])
            nc.sync.dma_start(out=Z0B[:N1, :], in_=Z0A[N1:, :])

            nc.sync.dma_start(out=ct["c_L"], in_=cd["c_L"].ap())
            if debug_dumps:
                nc.gpsimd.dma_start(out=dbg["dZ0A"].ap(), in_=Z0A)
                nc.gpsimd.dma_start(out=dbg["dZ0B"].ap(), in_=Z0B)
                nc.gpsimd.dma_start(out=dbg["dG1"].ap(), in_=G1)
                nc.gpsimd.dma_start(out=dbg["dG2"].ap(), in_=G2)

            # ================= inverse: per-f mult + stage-1 (fused add) ======
            # DT layout: [k2 | (b, ri, n1', f, c)] bf16
            DT = spool.tile([N2, BL * 2 * N1 * FIL * C], BF16, tag="DT")
            dtv = DT.rearrange("p (b ri n1 f c) -> p b ri n1 f c",
                               b=BL, ri=2, n1=N1, f=FIL)
            Z0Av = Z0A.rearrange("p (r k) -> p r k", k=N2)
            Z0Bv = Z0B.rearrange("p (r k) -> p r k", k=N2)
            bkv = bk0.rearrange("p (f b c) -> p f b c", f=FIL, b=BL)
            with tc.tile_pool(name="zt", bufs=2) as ztp, \
                 tc.tile_pool(name="invp", bufs=2, space="PSUM") as ips:
                for f in range(FIL):
                    g1 = G1[:, f * N2:(f + 1) * N2][:, None, :].broadcast_to(
                        [N1s, ROWS, N2])
                    g2 = G2[:, f * N2:(f + 1) * N2][:, None, :].broadcast_to(
                        [N1s, ROWS, N2])
                    zt1 = ztp.tile([N1s, RN], BF16, tag="zt1")
                    zt2 = ztp.tile([N1s, RN], BF16, tag="zt2")
                    z1v = zt1.rearrange("p (r k) -> p r k", k=N2)
                    z2v = zt2.rearrange("p (r k) -> p r k", k=N2)
                    nc.vector.tensor_tensor(out=z1v, in0=Z0Av, in1=g1, op=AL.mult)
                    nc.vector.tensor_tensor(out=z2v, in0=Z0Bv, in1=g2, op=AL.mult)
                    # bias folded into the k=0 bin of V (k1=0 real, k2=0)
                    z1k0 = zt1.rearrange("p (b c k) -> p b c k", b=BL, c=C)[0:1, :, :, 0]
                    nc.gpsimd.tensor_tensor(out=z1k0, in0=z1k0, in1=bkv[0:1, f],
                                            op=AL.add)
                    if debug_dumps and f == 0:
                        nc.sync.dma_start(out=dbg["dZT1"].ap(), in_=zt1)
                        nc.sync.dma_start(out=dbg["dZT2"].ap(), in_=zt2)
                    # stage-1 iFFT: contract k1; data as weights, k2 -> partitions
                    cps = ips.tile([N2, BL * C * N1s], F32, tag="cps")
                    for b in range(BL):
                        for c in range(C):
                            sl = cps[:, (b * C + c) * N1s:(b * C + c + 1) * N1s]
                            lhs1 = zt1[:, (b * C + c) * N2:(b * C + c + 1) * N2]
                            lhs2 = zt2[:, (b * C + c) * N2:(b * C + c + 1) * N2]
                            nc.tensor.matmul(sl, lhs1, ct["c_M3"],
                                             start=True, stop=False)
                            nc.tensor.matmul(sl, lhs2, ct["c_M3"],
                                             start=False, stop=True)
                    # evacuate [k2 | (b,c,ri,n1')] -> DT [k2 | b,ri,n1',f,c]
                    cpv = cps.rearrange("p (b c ri n1) -> p b ri n1 c", b=BL, c=C, ri=2)
                    eng = nc.vector if (f % 4 == 3) else nc.scalar
                    if eng is nc.scalar:
                        nc.scalar.copy(out=dtv[:, :, :, :, f, :], in_=cpv)
                    else:
                        nc.vector.tensor_copy(out=dtv[:, :, :, :, f, :], in_=cpv)

            if debug_dumps:
                nc.sync.dma_start(out=dbg["dDT"].ap(), in_=DT)
            # ================= inverse stage-2 (M4) + store =================
            NB = 16   # n1' per PSUM group (4 banks)
            dt4 = DT.rearrange("p (b ri n1 fc) -> p b ri n1 fc", b=BL, ri=2, n1=N1)
            with tc.tile_pool(name="yp", bufs=2, space="PSUM") as yps, \
                 tc.tile_pool(name="yev", bufs=2) as yp:
                for b in range(BL):
                    for g0 in range(0, N1, NB):
                        ypsum = yps.tile([N2, NB * FC], F32, tag="yps")
                        for j in range(NB):
                            n1p = g0 + j
                            lr = ct["c_L"][:, n1p * N2:(n1p + 1) * N2]
                            li = ct["c_L"][:, (N1 + n1p) * N2:(N1 + n1p + 1) * N2]
                            sl = ypsum[:, j * FC:(j + 1) * FC]
                            nc.tensor.matmul(sl, lr, dt4[:, b, 0, n1p, :],
                                             start=True, stop=False)
                            nc.tensor.matmul(sl, li, dt4[:, b, 1, n1p, :],
                                             start=False, stop=True)
                        yt = yp.tile([N2, NB * FC], F32, tag="yt")
                        nc.scalar.copy(out=yt, in_=ypsum)
                        nc.sync.dma_start(
                            out=out_d.ap()[b].rearrange(
                                "(n2 n1) fc -> n2 n1 fc", n1=N1)[:, g0:g0 + NB, :],
                            in_=yt.rearrange("p (j fc) -> p j fc", j=NB))

    nc.compile()
    return nc


def host_inputs(cfg, x_sh, w_real, w_imag, s, b):
    """Build the per-core in_map (numpy) for one core's batch shard."""
    import ml_dtypes
    cs = host_consts(cfg)
    N1, N2, T, FIL, C, BL = cfg.N1, cfg.N2, cfg.T, cfg.FIL, cfg.C, cfg.BL
    KH = FIL * N2 // 2
    f32 = np.float32
    # packed s: rows 0..63 hold (f,k2) cols 0..KH-1, rows 64..127 the rest
    S = np.broadcast_to(np.asarray(s, f32).reshape(FIL, 1), (FIL, N2)).reshape(-1)
    srepP = np.concatenate([
        np.broadcast_to(S[:KH], (N1, KH)),
        np.broadcast_to(S[KH:], (N1, KH))], axis=0).astype(f32).copy()
    # bias folded into the k=0 bin: V[k=0] += T * bias[f*C + c]
    bf = np.asarray(b, f32).reshape(FIL, C)
    bk0 = np.broadcast_to((T * bf)[:, None, :], (FIL, BL, C)).reshape(1, -1)
    m = {
        "xs": np.ascontiguousarray(x_sh, dtype=f32),
        "wr": np.ascontiguousarray(w_real, dtype=f32),
        "wi": np.ascontiguousarray(w_imag, dtype=f32),
        "srepP": srepP,
        "bk0": bk0.astype(f32).copy(),
    }
    for k, v in cs.items():
        if k in ("c_L", "c_M3"):
            m[k] = v.astype(ml_dtypes.bfloat16)
        else:
            m[k] = v
    return m


_NC_CACHE = {}


def kernel(x, w_real, w_imag, s, b):
    """Full-input entry point: shard over 8 cores, run, gather."""
    from concourse.bass_utils import run_bass_kernel_spmd
    cfg = FULL
    n_cores = 8
    key = "full"
    if key not in _NC_CACHE:
        _NC_CACHE[key] = build_nc(cfg)
    nc = _NC_CACHE[key]
    x = np.asarray(x, dtype=np.float32)
    w_real = np.asarray(w_real, dtype=np.float32)
    w_imag = np.asarray(w_imag, dtype=np.float32)
    s = np.asarray(s, dtype=np.float32)
    b = np.asarray(b, dtype=np.float32)
    in_maps = []
    for i in range(n_cores):
        x_sh = x[i * cfg.BL:(i + 1) * cfg.BL]
        in_maps.append(host_inputs(cfg, x_sh, w_real, w_imag, s, b))
    res = run_bass_kernel_spmd(nc, in_maps, core_ids=list(range(n_cores)))
    outs = [res.results[i]["out"] for i in range(n_cores)]
    return np.concatenate(outs, axis=0).astype(np.float32)


# revision 29
# speedup vs baseline: 1.3441x; 1.1320x over previous
"""Trainium2 Bass kernel: frequency-domain regularized (Wiener) deconvolution.

Reference computation (B=16, T=8192, C=8, FIL=16):
    h  = fft(w_real + i*w_imag)            # (FIL, T)
    g  = conj(h) / (|h|^2 + s)             # (FIL, T)
    xf = fft(x, axis=T)                    # per (b, c) row
    y  = real(ifft(xf[:,None,:,:] * g[None,:,None,:]))
    out = y -> (B, T, FIL*C) + bias

Sharding: data-parallel over batch across 8 cores (2 batches/core); filter
params replicated.  FFTs are 4-step Cooley-Tukey matmuls on the PE array
(T = N2*N1, N2=128, N1=64; n = n1 + N1*n2, k = k2 + N2*k1):

  forward:  M1 (contract n2, fp32r) -> twiddle W^(n1 k2) (DVE real half,
            Pool imag half) -> PE transpose T1 (4 per PSUM bank, ACT evac)
            -> M2 (contract n1, stacked-complex K) -> Z0A [k1r;k1i|(b,c,k2)]
  filter:   H-path processed in f-QUARTERS pipelined through (ACT stage,
            Pool/DVE twiddle, T1H, M2H, Hs evac); G pipeline runs twice on
            partition-packed [128, 512] halves covering f {0-3,8-11} then
            {4-7,12-15}; assembled into stacked bf16 G1=[Gr;Gr], G3=[Gi;Gi]
  inverse:  per (b,f) unit: zt1=Z0A_b*G1f, zt3=Z0A_b*G3f on DVE (some zt3
            on Pool); bias folded into the k=0 bin of zt1 (tiny DVE op);
            stage-1 iFFT contracts k1 with the DATA as matmul weights
            (out partitions = k2) and the complex add fused into PSUM
            accumulation via two weight matrices c_M3/c_M3p -> no PE
            transpose, no DVE add, no swapped Z0B copy; ACT evacuates
            [k2 | (ri,n1',f,c)] bf16 per unit; M4 contracts k2 per (b,n1')
            with inverse twiddle folded into static bf16 weights.  Units run
            b-major so M4(b0) overlaps batch-1's unit pipeline.
"""
import sys

sys.path.insert(0, "/opt/trn_rl_repo")

import numpy as np


def _get_cc():
    import concourse.bacc as bacc
    import concourse.mybir as mybir
    import concourse.tile as tile
    return bacc, mybir, tile


class Cfg:
    def __init__(self, T=8192, N2=128, N1=64, BL=2, C=8, FIL=16):
        assert N1 * N2 == T
        self.T, self.N2, self.N1, self.BL, self.C, self.FIL = T, N2, N1, BL, C, FIL
        self.ROWS = BL * C
        self.FC = FIL * C


FULL = Cfg()


def host_consts(cfg):
    """Static (input-independent) weights, as fp32 numpy arrays."""
    T, N1, N2 = cfg.T, cfg.N1, cfg.N2
    f32 = np.float32
    cs = {}
    a2 = np.arange(N2)
    a1 = np.arange(N1)
    F2 = np.exp(-2j * np.pi * np.outer(a2, a2) / N2)        # [n2, k2]
    cs["c_F2r"] = F2.real.astype(f32)
    cs["c_F2i"] = F2.imag.astype(f32)
    cs["c_F2in"] = (-F2.imag).astype(f32)
    Tw = np.exp(-2j * np.pi * np.outer(a2, a1) / T)         # [k2, n1]
    cs["c_Twr"] = Tw.real.astype(f32)
    cs["c_Twi"] = Tw.imag.astype(f32)
    cs["c_Twin"] = (-Tw.imag).astype(f32)
    F1 = np.exp(-2j * np.pi * np.outer(a1, a1) / N1)        # [n1, k1]
    cs["c_M2"] = np.hstack([np.vstack([F1.real, -F1.imag]),
                            np.vstack([F1.imag, F1.real])]).astype(f32)
    Fb1 = np.exp(2j * np.pi * np.outer(a1, a1) / N1)        # [k1, n1']
    M3 = np.hstack([np.vstack([Fb1.real, -Fb1.imag]),
                    np.vstack([Fb1.imag, Fb1.real])]).astype(f32)
    cs["c_M3"] = M3
    # row-swapped/sign-flipped variant: with zt3 = [Zr*gi; Zi*gi] where
    # gi = Hi*r = -Im(G),  zt3^T @ c_M3p == zt2^T @ c_M3 for the old
    # zt2 = [Zi*gi; -Zr*gi] (stacked-swap complex-multiply half)
    cs["c_M3p"] = np.vstack([-M3[N1:], M3[:N1]]).astype(f32)
    # M4 per-n1' weights, inverse twiddle folded in:
    #   L_{n1'}[k2, n2'] = exp(+2j pi k2 n2'/N2) * exp(+2j pi n1' k2 / T) / T
    Fb2 = np.exp(2j * np.pi * np.outer(a2, a2) / N2)        # [k2, n2']
    ph = np.exp(2j * np.pi * np.outer(a1, a2) / T)          # [n1', k2]
    L = Fb2[None, :, :] * ph[:, :, None] / T                # [n1', k2, n2']
    Lr = L.real.transpose(1, 0, 2).reshape(N2, N1 * N2)     # [k2, (n1', n2')]
    Lin = (-L.imag).transpose(1, 0, 2).reshape(N2, N1 * N2)
    cs["c_L"] = np.concatenate([Lr, Lin], axis=1).astype(f32)  # [k2 | (ri, n1', n2')]
    cs["c_idr"] = np.eye(N2, dtype=f32)
    return cs


# f-quarters: packed-G half 0 covers quarters (0, 2) = f {0..3, 8..11}
QGROUPS = [(0, 2), (1, 3)]
FORDER = [0, 1, 2, 3, 8, 9, 10, 11, 4, 5, 6, 7, 12, 13, 14, 15]


def build_nc(cfg, debug_dumps=False):
    bacc, mybir, tile = _get_cc()
    F32, F32R, BF16 = mybir.dt.float32, mybir.dt.float32r, mybir.dt.bfloat16
    AL = mybir.AluOpType
    T, N1, N2, BL, C, FIL = cfg.T, cfg.N1, cfg.N2, cfg.BL, cfg.C, cfg.FIL
    ROWS, FC = cfg.ROWS, cfg.FC
    N1s = 2 * N1          # stacked (real; imag) partition dim = 128
    KF = FIL * N2         # H/G free size, (f, k2) order = 2048
    KH = KF // 2          # packed layout free size = 1024
    KQ = KF // 4          # one f-quarter = 512
    RN = ROWS * N2        # Z0 free size, (b, c, k2) order = 2048
    KB = C * N2           # per-(b,f) free size = 1024
    MCH = 512

    nc = bacc.Bacc("TRN2", debug=False)

    xs_d = nc.dram_tensor("xs", [BL, T, C], F32R, kind="ExternalInput")
    wr_d = nc.dram_tensor("wr", [FIL, T], F32R, kind="ExternalInput")
    wi_d = nc.dram_tensor("wi", [FIL, T], F32R, kind="ExternalInput")
    srepP_d = nc.dram_tensor("srepP", [N2, KH], F32, kind="ExternalInput")
    bk0_d = nc.dram_tensor("bk0", [1, FIL * BL * C], F32, kind="ExternalInput")
    cdef = [
        ("c_F2r", [N2, N2], F32R), ("c_F2i", [N2, N2], F32R), ("c_F2in", [N2, N2], F32R),
        ("c_Twr", [N2, N1], F32), ("c_Twi", [N2, N1], F32), ("c_Twin", [N2, N1], F32),
        ("c_M2", [N1s, N1s], F32R), ("c_M3", [N1s, N1s], BF16),
        ("c_M3p", [N1s, N1s], BF16),
        ("c_L", [N2, 2 * N1 * N2], BF16),
        ("c_idr", [N2, N2], F32R),
    ]
    cd = {}
    for name, shape, dt_ in cdef:
        cd[name] = nc.dram_tensor(name, shape, dt_, kind="ExternalInput")
    out_d = nc.dram_tensor("out", [BL, T, FC], F32, kind="ExternalOutput")
    dbg = {}
    if debug_dumps:
        for nm, shape, ddt in [("dZ0A", [N1s, RN], BF16),
                               ("dG1", [N1s, KF], BF16), ("dG3", [N1s, KF], BF16),
                               ("dZT1", [N1s, KB], BF16), ("dZT3", [N1s, KB], BF16),
                               ("dHs", [N1s, KF], F32),
                               ("dDT", [N2, BL * 2 * N1 * FIL * C], BF16)]:
            dbg[nm] = nc.dram_tensor(nm, shape, ddt, kind="ExternalOutput")

    with tile.TileContext(nc) as tc:
        with tc.tile_pool(name="consts", bufs=1) as cpool, \
             tc.tile_pool(name="spec", bufs=1) as spool:
            ct = {}
            for name, shape, dt_ in cdef:
                t_ = cpool.tile(shape, dt_, tag=name)
                if name != "c_L":
                    nc.sync.dma_start(out=t_, in_=cd[name].ap())
                ct[name] = t_
            bk0 = cpool.tile([1, FIL * BL * C], F32, tag="bk0")
            nc.sync.dma_start(out=bk0, in_=bk0_d.ap())
            srepP = cpool.tile([N2, KH], F32, tag="srepP")
            nc.sync.dma_start(out=srepP, in_=srepP_d.ap())

            Z0A = spool.tile([N1s, RN], BF16, tag="Z0A")   # [k1r;k1i | (b,c,k2)]
            G1 = spool.tile([N1s, KF], BF16, tag="G1")     # [Gr;Gr | (f,k2)]
            G3 = spool.tile([N1s, KF], BF16, tag="G3")     # [Gi;Gi | (f,k2)]
            BT = spool.tile([N1s, RN], F32R, tag="BT")     # [n1r;n1i | (b,c,k2)]

            # ============ H forward (pipelined in f-quarters) + x forward ====
            with tc.tile_pool(name="fh", bufs=1) as hp, \
                 tc.tile_pool(name="fx", bufs=1) as fp, \
                 tc.tile_pool(name="gp", bufs=1) as gp:
                wtr = hp.tile([N2, FIL * N1], F32R, tag="wtr")
                wti = hp.tile([N2, FIL * N1], F32R, tag="wti")
                nc.sync.dma_start(out=wtr.rearrange("p (f n) -> p f n", f=FIL),
                                  in_=wr_d.ap().rearrange("f (p n) -> p f n", p=N2))
                nc.sync.dma_start(out=wti.rearrange("p (f n) -> p f n", f=FIL),
                                  in_=wi_d.ap().rearrange("f (p n) -> p f n", p=N2))
                xts = []
                for b in range(BL):
                    xt = fp.tile([N2, N1 * C], F32R, tag=f"xt{b}")
                    nc.sync.dma_start(
                        out=xt, in_=xs_d.ap()[b].rearrange("(p q) c -> p (q c)", p=N2))
                    xts.append(xt)

                xps = []
                with tc.tile_pool(name="fxp", bufs=1, space="PSUM") as fps, \
                     tc.tile_pool(name="fhp", bufs=1, space="PSUM") as hps:
                    # --- PE: H-M1 (all quarters), then x-M1 (both b) ---
                    hps_t = hps.tile([N2, 2 * FIL * N1], F32, tag="Hps")
                    for c0, c1 in chunks_of(FIL * N1, MCH):
                        nc.tensor.matmul(hps_t[:, c0:c1], ct["c_F2r"], wtr[:, c0:c1],
                                         start=True, stop=False)
                        nc.tensor.matmul(hps_t[:, c0:c1], ct["c_F2in"], wti[:, c0:c1],
                                         start=False, stop=True)
                        d0 = FIL * N1
                        nc.tensor.matmul(hps_t[:, d0 + c0:d0 + c1], ct["c_F2i"],
                                         wtr[:, c0:c1], start=True, stop=False)
                        nc.tensor.matmul(hps_t[:, d0 + c0:d0 + c1], ct["c_F2r"],
                                         wti[:, c0:c1], start=False, stop=True)
                    for b in range(BL):
                        ps = fps.tile([N2, 2 * N1 * C], F32, tag=f"Aps{b}")
                        for comp, w in ((0, "c_F2r"), (1, "c_F2i")):
                            for c0, c1 in chunks_of(N1 * C, MCH):
                                nc.tensor.matmul(
                                    ps[:, comp * N1 * C + c0: comp * N1 * C + c1],
                                    ct[w], xts[b][:, c0:c1], start=True, stop=True)
                        xps.append(ps)

                    # --- x twiddle: real half DVE (reads PSUM), imag Pool ---
                    Hsb = hp.tile([N2, 2 * FIL * N1], F32, tag="Hsb")
                    BHc = hp.tile([N2, FIL * 2 * N1], F32R, tag="BHc")
                    Bcs = []
                    for b in range(BL):
                        ps = xps[b]
                        Ar = ps[:, :N1 * C].rearrange("p (n c) -> p n c", c=C)
                        Ai = ps[:, N1 * C:].rearrange("p (n c) -> p n c", c=C)
                        Bc = fp.tile([N2, 2 * N1 * C], F32R, tag=f"Bc{b}")
                        Bcs.append(Bc)
                        u = fp.tile([N2, N1 * C], F32, tag="u")
                        v = fp.tile([N2, N1 * C], F32, tag="v")

                        def bcx(w):
                            return ct[w][:, :, None].broadcast_to([N2, N1, C])

                        uv = u.rearrange("p (n c) -> p n c", c=C)
                        vv = v.rearrange("p (n c) -> p n c", c=C)
                        Brv = Bc[:, :N1 * C].rearrange("p (n c) -> p n c", c=C)
                        Biv = Bc[:, N1 * C:].rearrange("p (n c) -> p n c", c=C)
                        u2 = fp.tile([N2, N1 * C], F32, tag="u2")
                        v2_ = fp.tile([N2, N1 * C], F32, tag="v2")
                        u2v = u2.rearrange("p (n c) -> p n c", c=C)
                        v2v = v2_.rearrange("p (n c) -> p n c", c=C)
                        # Pool cannot read PSUM: stage A via ACT for imag half
                        Asb = fp.tile([N2, 2 * N1 * C], F32, tag=f"Asb{b}")
                        nc.scalar.copy(out=Asb, in_=ps)
                        Asr = Asb[:, :N1 * C].rearrange("p (n c) -> p n c", c=C)
                        Asi = Asb[:, N1 * C:].rearrange("p (n c) -> p n c", c=C)
                        nc.vector.tensor_tensor(out=uv, in0=Ar, in1=bcx("c_Twr"),
                                                op=AL.mult)
                        nc.vector.tensor_tensor(out=vv, in0=Ai, in1=bcx("c_Twin"),
                                                op=AL.mult)
                        nc.vector.tensor_tensor(out=Brv, in0=uv, in1=vv, op=AL.add)
                        nc.gpsimd.tensor_tensor(out=u2v, in0=Asr, in1=bcx("c_Twi"),
                                                op=AL.mult)
                        nc.gpsimd.tensor_tensor(out=v2v, in0=Asi, in1=bcx("c_Twr"),
                                                op=AL.mult)
                        nc.gpsimd.tensor_tensor(out=Biv, in0=u2v, in1=v2v, op=AL.add)

                    # --- H twiddle per quarter: real DVE (PSUM), imag Pool ---
                    Q = FIL // 4
                    for q in range(4):
                        fsl = slice(q * Q * N1, (q + 1) * Q * N1)
                        nc.scalar.copy(out=Hsb[:, q * Q * N1:(q + 1) * Q * N1],
                                       in_=hps_t[:, q * Q * N1:(q + 1) * Q * N1])
                        nc.scalar.copy(
                            out=Hsb[:, FIL * N1 + q * Q * N1: FIL * N1 + (q + 1) * Q * N1],
                            in_=hps_t[:, FIL * N1 + q * Q * N1: FIL * N1 + (q + 1) * Q * N1])
                        Arq = hps_t[:, :FIL * N1][:, fsl].rearrange(
                            "p (f n) -> p f n", f=Q)
                        Aiq = hps_t[:, FIL * N1:][:, fsl].rearrange(
                            "p (f n) -> p f n", f=Q)
                        Asrq = Hsb[:, :FIL * N1][:, fsl].rearrange(
                            "p (f n) -> p f n", f=Q)
                        Asiq = Hsb[:, FIL * N1:][:, fsl].rearrange(
                            "p (f n) -> p f n", f=Q)

                        def bchq(w):
                            return ct[w][:, None, :].broadcast_to([N2, Q, N1])

                        uhq = hp.tile([N2, Q * N1], F32, tag="uh")
                        vhq = hp.tile([N2, Q * N1], F32, tag="vh")
                        u2hq = hp.tile([N2, Q * N1], F32, tag="u2h")
                        v2hq = hp.tile([N2, Q * N1], F32, tag="v2h")
                        uvq = uhq.rearrange("p (f n) -> p f n", f=Q)
                        vvq = vhq.rearrange("p (f n) -> p f n", f=Q)
                        u2vq = u2hq.rearrange("p (f n) -> p f n", f=Q)
                        v2vq = v2hq.rearrange("p (f n) -> p f n", f=Q)
                        BHq = BHc[:, 2 * q * Q * N1:2 * (q + 1) * Q * N1].rearrange(
                            "p (f m n) -> p f m n", f=Q, m=2)
                        nc.vector.tensor_tensor(out=uvq, in0=Arq, in1=bchq("c_Twr"),
                                                op=AL.mult)
                        nc.vector.tensor_tensor(out=vvq, in0=Aiq, in1=bchq("c_Twin"),
                                                op=AL.mult)
                        nc.vector.tensor_tensor(out=BHq[:, :, 0, :], in0=uvq, in1=vvq,
                                                op=AL.add)
                        nc.gpsimd.tensor_tensor(out=u2vq, in0=Asrq, in1=bchq("c_Twi"),
                                                op=AL.mult)
                        nc.gpsimd.tensor_tensor(out=v2vq, in0=Asiq, in1=bchq("c_Twr"),
                                                op=AL.mult)
                        nc.gpsimd.tensor_tensor(out=BHq[:, :, 1, :], in0=u2vq,
                                                in1=v2vq, op=AL.add)

                # --- x T1 transposes (4 per bank) + evac; needs Bc done ---
                with tc.tile_pool(name="t1p", bufs=2, space="PSUM") as t1ps:
                    for b in range(BL):
                        Bview = Bcs[b].rearrange("p (m n c) -> p m n c", m=2, c=C)
                        for qq in range(C // 4):
                            tp = t1ps.tile([N1s, 4 * N2], F32R, tag="t1")
                            for j in range(4):
                                c = 4 * qq + j
                                nc.tensor.transpose(tp[:, j * N2:(j + 1) * N2],
                                                    Bview[:, :, :, c], ct["c_idr"])
                            row = b * C + 4 * qq
                            nc.scalar.copy(out=BT[:, row * N2:(row + 4) * N2], in_=tp)

                    # --- H T1H + M2H + Hs evac per quarter ---
                    Hs = hp.tile([N1s, KF], F32, tag="Hs")
                    with tc.tile_pool(name="m2hp", bufs=2, space="PSUM") as m2hps, \
                         tc.tile_pool(name="t1hp", bufs=2, space="PSUM") as t1hps:
                        for q in range(4):
                            tp = t1hps.tile([N1s, 4 * N2], F32R, tag="t1h")
                            for j in range(4):
                                f = 4 * q + j
                                nc.tensor.transpose(
                                    tp[:, j * N2:(j + 1) * N2],
                                    BHc[:, f * 2 * N1:(f + 1) * 2 * N1], ct["c_idr"])
                            BTHq = hp.tile([N1s, 4 * N2], F32R, tag="BTH")
                            nc.scalar.copy(out=BTHq, in_=tp)
                            psq = m2hps.tile([N1s, 4 * N2], F32, tag="m2h")
                            nc.tensor.matmul(psq, ct["c_M2"], BTHq,
                                             start=True, stop=True)
                            nc.scalar.copy(out=Hs[:, q * KQ:(q + 1) * KQ], in_=psq)

                    # --- M2 for x + Z0A evac ---
                    with tc.tile_pool(name="m2p", bufs=1, space="PSUM") as m2ps:
                        psx = m2ps.tile([N1s, RN], F32, tag="m2")
                        for c0, c1 in chunks_of(RN, MCH):
                            nc.tensor.matmul(psx[:, c0:c1], ct["c_M2"], BT[:, c0:c1],
                                             start=True, stop=True)
                        nc.vector.tensor_copy(out=Z0A, in_=psx)

                # ---- G pipeline per packed half h: quarters (h, h+2) ----
                # packed rows 0..63 <- quarter h (f h*4..h*4+3),
                #        rows 64..127 <- quarter h+2 (f h*4+8..h*4+11)
                for h in range(2):
                    qa, qb = QGROUPS[h]
                    HrP = gp.tile([N2, KQ], F32, tag="HrP")
                    HiP = gp.tile([N2, KQ], F32, tag="HiP")
                    nc.sync.dma_start(out=HrP[:N1, :], in_=Hs[:N1, qa * KQ:(qa + 1) * KQ])
                    nc.sync.dma_start(out=HrP[N1:, :], in_=Hs[:N1, qb * KQ:(qb + 1) * KQ])
                    nc.sync.dma_start(out=HiP[:N1, :], in_=Hs[N1:, qa * KQ:(qa + 1) * KQ])
                    nc.sync.dma_start(out=HiP[N1:, :], in_=Hs[N1:, qb * KQ:(qb + 1) * KQ])
                    sq1 = gp.tile([N2, KQ], F32, tag="sq1")
                    sq2 = gp.tile([N2, KQ], F32, tag="sq2")
                    nc.scalar.square(sq1, HrP)
                    nc.gpsimd.tensor_tensor(out=sq2, in0=HiP, in1=HiP, op=AL.mult)
                    nc.gpsimd.tensor_tensor(out=sq2, in0=sq1, in1=sq2, op=AL.add)
                    srp = srepP[:, h * KQ:(h + 1) * KQ]
                    nc.vector.tensor_tensor(out=sq2, in0=sq2, in1=srp, op=AL.add)
                    r = sq1
                    nc.vector.reciprocal(out=r, in_=sq2)
                    GrPb = gp.tile([N2, KQ], BF16, tag="GrPb")
                    GiPb = gp.tile([N2, KQ], BF16, tag="GiPb")
                    nc.vector.tensor_tensor(out=GrPb, in0=HrP, in1=r, op=AL.mult)
                    nc.vector.tensor_tensor(out=GiPb, in0=HiP, in1=r, op=AL.mult)
                    # unpack to stacked [Gr;Gr] / [Gi;Gi]; Im G = -Hi*r is
                    # handled by the c_M3p sign structure, so G3 holds +Hi*r
                    # with the sign flip folded into c_M3p... (see below)
                    for (src, dstt) in ((GrPb, G1), (GiPb, G3)):
                        nc.sync.dma_start(out=dstt[:N1, qa * KQ:(qa + 1) * KQ],
                                          in_=src[:N1, :])
                        nc.sync.dma_start(out=dstt[:N1, qb * KQ:(qb + 1) * KQ],
                                          in_=src[N1:, :])
                        nc.sync.dma_start(out=dstt[N1:, qa * KQ:(qa + 1) * KQ],
                                          in_=src[:N1, :])
                        nc.sync.dma_start(out=dstt[N1:, qb * KQ:(qb + 1) * KQ],
                                          in_=src[N1:, :])

            if debug_dumps:
                nc.gpsimd.dma_start(out=dbg["dZ0A"].ap(), in_=Z0A)
                nc.gpsimd.dma_start(out=dbg["dG1"].ap(), in_=G1)
                nc.gpsimd.dma_start(out=dbg["dG3"].ap(), in_=G3)
                nc.sync.dma_start(out=dbg["dHs"].ap(), in_=Hs)

            nc.sync.dma_start(out=ct["c_L"], in_=cd["c_L"].ap())

            # ================= inverse units + M4, b-major =================
            DT = spool.tile([N2, BL * 2 * N1 * FIL * C], BF16, tag="DT")
            dtv = DT.rearrange("p (b ri n1 f c) -> p b ri n1 f c",
                               b=BL, ri=2, n1=N1, f=FIL)
            dt4 = DT.rearrange("p (b ri n1 fc) -> p b ri n1 fc", b=BL, ri=2, n1=N1)
            bkv = bk0.rearrange("p (f b c) -> p f b c", f=FIL, b=BL)
            NB = 8   # n1' per M4 PSUM group (2 banks)
            with tc.tile_pool(name="zt", bufs=3) as ztp, \
                 tc.tile_pool(name="invp", bufs=2, space="PSUM") as ips, \
                 tc.tile_pool(name="yp", bufs=2, space="PSUM") as yps, \
                 tc.tile_pool(name="yev", bufs=2) as yp:
                for b in range(BL):
                    for fi, f in enumerate(FORDER):
                        zb = Z0A[:, b * KB:(b + 1) * KB].rearrange(
                            "p (c k) -> p c k", c=C)
                        g1 = G1[:, f * N2:(f + 1) * N2][:, None, :].broadcast_to(
                            [N1s, C, N2])
                        g3 = G3[:, f * N2:(f + 1) * N2][:, None, :].broadcast_to(
                            [N1s, C, N2])
                        zt1 = ztp.tile([N1s, KB], BF16, tag="zt1")
                        zt3 = ztp.tile([N1s, KB], BF16, tag="zt3")
                        z1v = zt1.rearrange("p (c k) -> p c k", c=C)
                        z3v = zt3.rearrange("p (c k) -> p c k", c=C)
                        nc.vector.tensor_tensor(out=z1v, in0=zb, in1=g1, op=AL.mult)
                        meng = nc.gpsimd if fi % 4 == 2 else nc.vector
                        meng.tensor_tensor(out=z3v, in0=zb, in1=g3, op=AL.mult)
                        # bias into the k=0 bin (k1=0 real, k2=0) of zt1
                        z1k0 = zt1.rearrange("p (c k) -> p c k", c=C)[0:1, :, 0]
                        nc.vector.tensor_tensor(out=z1k0, in0=z1k0,
                                                in1=bkv[0:1, f, b], op=AL.add)
                        if debug_dumps and f == 0 and b == 0:
                            nc.sync.dma_start(out=dbg["dZT1"].ap(), in_=zt1)
                            nc.sync.dma_start(out=dbg["dZT3"].ap(), in_=zt3)
                        cps = ips.tile([N2, C * N1s], F32, tag="cps")
                        for c in range(C):
                            sl = cps[:, c * N1s:(c + 1) * N1s]
                            nc.tensor.matmul(sl, zt1[:, c * N2:(c + 1) * N2],
                                             ct["c_M3"], start=True, stop=False)
                            nc.tensor.matmul(sl, zt3[:, c * N2:(c + 1) * N2],
                                             ct["c_M3p"], start=False, stop=True)
                        cpv = cps.rearrange("p (c ri n1) -> p ri n1 c", c=C, ri=2)
                        nc.scalar.copy(out=dtv[:, b, :, :, f, :], in_=cpv)
                    if debug_dumps and b == 0:
                        nc.gpsimd.dma_start(out=dbg["dDT"].ap(), in_=DT)
                    # ---- M4 for this batch (overlaps next batch's units) ----
                    for g0 in range(0, N1, NB):
                        ypsum = yps.tile([N2, NB * FC], F32, tag="yps")
                        for j in range(NB):
                            n1p = g0 + j
                            lr = ct["c_L"][:, n1p * N2:(n1p + 1) * N2]
                            li = ct["c_L"][:, (N1 + n1p) * N2:(N1 + n1p + 1) * N2]
                            sl = ypsum[:, j * FC:(j + 1) * FC]
                            nc.tensor.matmul(sl, lr, dt4[:, b, 0, n1p, :],
                                             start=True, stop=False)
                            nc.tensor.matmul(sl, li, dt4[:, b, 1, n1p, :],
                                             start=False, stop=True)
                        yt = yp.tile([N2, NB * FC], F32, tag="yt")
                        nc.scalar.copy(out=yt, in_=ypsum)
                        nc.sync.dma_start(
                            out=out_d.ap()[b].rearrange(
                                "(n2 n1) fc -> n2 n1 fc", n1=N1)[:, g0:g0 + NB, :],
                            in_=yt.rearrange("p (j fc) -> p j fc", j=NB))

    nc.compile()
    return nc


def chunks_of(total, step):
    return [(c0, min(total, c0 + step)) for c0 in range(0, total, step)]


def host_inputs(cfg, x_sh, w_real, w_imag, s, b):
    """Build the per-core in_map (numpy) for one core's batch shard."""
    import ml_dtypes
    cs = host_consts(cfg)
    N1, N2, T, FIL, C, BL = cfg.N1, cfg.N2, cfg.T, cfg.FIL, cfg.C, cfg.BL
    KQ = FIL * N2 // 4
    f32 = np.float32
    # packed s matching QGROUPS: half h rows 0..63 = quarter 2h? see QGROUPS
    S = np.broadcast_to(np.asarray(s, f32).reshape(FIL, 1), (FIL, N2)).reshape(-1)
    halves = []
    for (qa, qb) in QGROUPS:
        halves.append(np.concatenate([
            np.broadcast_to(S[qa * KQ:(qa + 1) * KQ], (N1, KQ)),
            np.broadcast_to(S[qb * KQ:(qb + 1) * KQ], (N1, KQ))], axis=0))
    srepP = np.concatenate(halves, axis=1).astype(f32).copy()
    bf = np.asarray(b, f32).reshape(FIL, C)
    bk0 = np.broadcast_to((T * bf)[:, None, :], (FIL, BL, C)).reshape(1, -1)
    m = {
        "xs": np.ascontiguousarray(x_sh, dtype=f32),
        "wr": np.ascontiguousarray(w_real, dtype=f32),
        "wi": np.ascontiguousarray(w_imag, dtype=f32),
        "srepP": srepP,
        "bk0": bk0.astype(f32).copy(),
    }
    for k, v in cs.items():
        if k in ("c_L", "c_M3", "c_M3p"):
            m[k] = v.astype(ml_dtypes.bfloat16)
        else:
            m[k] = v
    return m


_NC_CACHE = {}


def kernel(x, w_real, w_imag, s, b):
    """Full-input entry point: shard over 8 cores, run, gather."""
    from concourse.bass_utils import run_bass_kernel_spmd
    cfg = FULL
    n_cores = 8
    key = "full"
    if key not in _NC_CACHE:
        _NC_CACHE[key] = build_nc(cfg)
    nc = _NC_CACHE[key]
    x = np.asarray(x, dtype=np.float32)
    w_real = np.asarray(w_real, dtype=np.float32)
    w_imag = np.asarray(w_imag, dtype=np.float32)
    s = np.asarray(s, dtype=np.float32)
    b = np.asarray(b, dtype=np.float32)
    in_maps = []
    for i in range(n_cores):
        x_sh = x[i * cfg.BL:(i + 1) * cfg.BL]
        in_maps.append(host_inputs(cfg, x_sh, w_real, w_imag, s, b))
    res = run_bass_kernel_spmd(nc, in_maps, core_ids=list(range(n_cores)))
    outs = [res.results[i]["out"] for i in range(n_cores)]
    return np.concatenate(outs, axis=0).astype(np.float32)


# revision 33
# speedup vs baseline: 1.4821x; 1.1026x over previous
"""Trainium2 Bass kernel: frequency-domain regularized (Wiener) deconvolution.

Reference computation (B=16, T=8192, C=8, FIL=16):
    h  = fft(w_real + i*w_imag)            # (FIL, T)
    g  = conj(h) / (|h|^2 + s)             # (FIL, T)
    xf = fft(x, axis=T)                    # per (b, c) row
    y  = real(ifft(xf[:,None,:,:] * g[None,:,None,:]))
    out = y -> (B, T, FIL*C) + bias

Sharding: data-parallel over batch across 8 cores (2 batches/core); filter
params replicated.  FFTs are 4-step Cooley-Tukey matmuls on the PE array
(T = N2*N1, N2=128, N1=64; n = n1 + N1*n2, k = k2 + N2*k1):

  forward:  M1 (contract n2, fp32r) -> twiddle W^(n1 k2) (DVE real half,
            Pool imag half) -> PE transpose T1 (4 per PSUM bank, ACT evac)
            -> M2 (contract n1, stacked-complex K) -> Z0A [k1r;k1i|(b,c,k2)]
  filter:   H-path processed in f-QUARTERS pipelined through (ACT stage,
            Pool/DVE twiddle, T1H, M2H, Hs evac); G pipeline runs twice on
            partition-packed [128, 512] halves covering f {0-3,8-11} then
            {4-7,12-15}; assembled into stacked bf16 G1=[Gr;Gr], G3=[Gi;Gi]
  inverse:  per (b,f) unit: zt1=Z0A_b*G1f, zt3=Z0A_b*G3f on DVE (some zt3
            on Pool); bias folded into the k=0 bin of zt1 (tiny DVE op);
            stage-1 iFFT contracts k1 with the DATA as matmul weights
            (out partitions = k2) and the complex add fused into PSUM
            accumulation via two weight matrices c_M3/c_M3p -> no PE
            transpose, no DVE add, no swapped Z0B copy; ACT evacuates
            [k2 | (ri,n1',f,c)] bf16 per unit; M4 contracts k2 per (b,n1')
            with inverse twiddle folded into static bf16 weights.  Units run
            b-major so M4(b0) overlaps batch-1's unit pipeline.
"""
import sys

sys.path.insert(0, "/opt/trn_rl_repo")

import numpy as np


def _get_cc():
    import concourse.bacc as bacc
    import concourse.mybir as mybir
    import concourse.tile as tile
    return bacc, mybir, tile


class Cfg:
    def __init__(self, T=8192, N2=128, N1=64, BL=2, C=8, FIL=16):
        assert N1 * N2 == T
        self.T, self.N2, self.N1, self.BL, self.C, self.FIL = T, N2, N1, BL, C, FIL
        self.ROWS = BL * C
        self.FC = FIL * C


FULL = Cfg()


def host_consts(cfg):
    """Static (input-independent) weights, as fp32 numpy arrays."""
    T, N1, N2 = cfg.T, cfg.N1, cfg.N2
    f32 = np.float32
    cs = {}
    a2 = np.arange(N2)
    a1 = np.arange(N1)
    F2 = np.exp(-2j * np.pi * np.outer(a2, a2) / N2)        # [n2, k2]
    cs["c_F2r"] = F2.real.astype(f32)
    cs["c_F2i"] = F2.imag.astype(f32)
    cs["c_F2in"] = (-F2.imag).astype(f32)
    Tw = np.exp(-2j * np.pi * np.outer(a2, a1) / T)         # [k2, n1]
    cs["c_Twr"] = Tw.real.astype(f32)
    cs["c_Twi"] = Tw.imag.astype(f32)
    cs["c_Twin"] = (-Tw.imag).astype(f32)
    F1 = np.exp(-2j * np.pi * np.outer(a1, a1) / N1)        # [n1, k1]
    cs["c_M2"] = np.hstack([np.vstack([F1.real, -F1.imag]),
                            np.vstack([F1.imag, F1.real])]).astype(f32)
    Fb1 = np.exp(2j * np.pi * np.outer(a1, a1) / N1)        # [k1, n1']
    M3 = np.hstack([np.vstack([Fb1.real, -Fb1.imag]),
                    np.vstack([Fb1.imag, Fb1.real])]).astype(f32)
    cs["c_M3"] = M3
    # row-swapped/sign-flipped variant: with zt3 = [Zr*gi; Zi*gi] where
    # gi = Hi*r = -Im(G),  zt3^T @ c_M3p == zt2^T @ c_M3 for the old
    # zt2 = [Zi*gi; -Zr*gi] (stacked-swap complex-multiply half)
    cs["c_M3p"] = np.vstack([-M3[N1:], M3[:N1]]).astype(f32)
    # M4 per-n1' weights, inverse twiddle folded in:
    #   L_{n1'}[k2, n2'] = exp(+2j pi k2 n2'/N2) * exp(+2j pi n1' k2 / T) / T
    Fb2 = np.exp(2j * np.pi * np.outer(a2, a2) / N2)        # [k2, n2']
    ph = np.exp(2j * np.pi * np.outer(a1, a2) / T)          # [n1', k2]
    L = Fb2[None, :, :] * ph[:, :, None] / T                # [n1', k2, n2']
    Lr = L.real.transpose(1, 0, 2).reshape(N2, N1 * N2)     # [k2, (n1', n2')]
    Lin = (-L.imag).transpose(1, 0, 2).reshape(N2, N1 * N2)
    cs["c_L"] = np.concatenate([Lr, Lin], axis=1).astype(f32)  # [k2 | (ri, n1', n2')]
    cs["c_idb"] = np.eye(N2, dtype=f32)
    return cs


# f-quarters: packed-G half 0 covers quarters (0, 2) = f {0..3, 8..11}
QGROUPS = [(0, 2), (1, 3)]
FORDER = [0, 1, 2, 3, 8, 9, 10, 11, 4, 5, 6, 7, 12, 13, 14, 15]


def build_nc(cfg, debug_dumps=False):
    bacc, mybir, tile = _get_cc()
    F32, F32R, BF16 = mybir.dt.float32, mybir.dt.float32r, mybir.dt.bfloat16
    AL = mybir.AluOpType
    T, N1, N2, BL, C, FIL = cfg.T, cfg.N1, cfg.N2, cfg.BL, cfg.C, cfg.FIL
    ROWS, FC = cfg.ROWS, cfg.FC
    N1s = 2 * N1          # stacked (real; imag) partition dim = 128
    KF = FIL * N2         # H/G free size, (f, k2) order = 2048
    KH = KF // 2          # packed layout free size = 1024
    KQ = KF // 4          # one f-quarter = 512
    RN = ROWS * N2        # Z0 free size, (b, c, k2) order = 2048
    KB = C * N2           # per-(b,f) free size = 1024
    MCH = 512

    nc = bacc.Bacc("TRN2", debug=False)

    xs_d = nc.dram_tensor("xs", [BL, T, C], F32R, kind="ExternalInput")
    wr_d = nc.dram_tensor("wr", [FIL, T], F32R, kind="ExternalInput")
    wi_d = nc.dram_tensor("wi", [FIL, T], F32R, kind="ExternalInput")
    srepP_d = nc.dram_tensor("srepP", [N2, KH], F32, kind="ExternalInput")
    bk0_d = nc.dram_tensor("bk0", [1, FIL * BL * C], F32, kind="ExternalInput")
    cdef = [
        ("c_F2r", [N2, N2], F32R), ("c_F2i", [N2, N2], F32R), ("c_F2in", [N2, N2], F32R),
        ("c_Twr", [N2, N1], BF16), ("c_Twi", [N2, N1], BF16), ("c_Twin", [N2, N1], BF16),
        ("c_M2", [N1s, N1s], BF16), ("c_M3", [N1s, N1s], BF16),
        ("c_M3p", [N1s, N1s], BF16),
        ("c_L", [N2, 2 * N1 * N2], BF16),
        ("c_idb", [N2, N2], BF16),
    ]
    cd = {}
    for name, shape, dt_ in cdef:
        cd[name] = nc.dram_tensor(name, shape, dt_, kind="ExternalInput")
    out_d = nc.dram_tensor("out", [BL, T, FC], F32, kind="ExternalOutput")
    dbg = {}
    if debug_dumps:
        for nm, shape, ddt in [("dZ0A", [N1s, RN], BF16),
                               ("dG1", [N1s, KF], BF16), ("dG3", [N1s, KF], BF16),
                               ("dZT1", [N1s, KB], BF16), ("dZT3", [N1s, KB], BF16),
                               ("dHs", [N1s, KF], F32),
                               ("dDT", [N2, BL * 2 * N1 * FIL * C], BF16)]:
            dbg[nm] = nc.dram_tensor(nm, shape, ddt, kind="ExternalOutput")

    with tile.TileContext(nc) as tc:
        with tc.tile_pool(name="consts", bufs=1) as cpool, \
             tc.tile_pool(name="spec", bufs=1) as spool:
            ct = {}
            for name, shape, dt_ in cdef:
                t_ = cpool.tile(shape, dt_, tag=name)
                if name != "c_L":
                    nc.sync.dma_start(out=t_, in_=cd[name].ap())
                ct[name] = t_
            bk0 = cpool.tile([1, FIL * BL * C], F32, tag="bk0")
            nc.sync.dma_start(out=bk0, in_=bk0_d.ap())
            srepP = cpool.tile([N2, KH], F32, tag="srepP")
            nc.sync.dma_start(out=srepP, in_=srepP_d.ap())

            Z0A = spool.tile([N1s, RN], BF16, tag="Z0A")   # [k1r;k1i | (b,c,k2)]
            G1 = spool.tile([N1s, KF], BF16, tag="G1")     # [Gr;Gr | (f,k2)]
            G3 = spool.tile([N1s, KF], BF16, tag="G3")     # [Gi;Gi | (f,k2)]
            BT = spool.tile([N1s, RN], BF16, tag="BT")     # [n1r;n1i | (b,c,k2)]

            # ============ H forward (f-quarters) + x forward, interleaved ===
            with tc.tile_pool(name="fh", bufs=1) as hp, \
                 tc.tile_pool(name="fx", bufs=1) as fp, \
                 tc.tile_pool(name="gp", bufs=1) as gp:
                wtr = hp.tile([N2, FIL * N1], F32R, tag="wtr")
                wti = hp.tile([N2, FIL * N1], F32R, tag="wti")
                nc.sync.dma_start(out=wtr.rearrange("p (f n) -> p f n", f=FIL),
                                  in_=wr_d.ap().rearrange("f (p n) -> p f n", p=N2))
                nc.sync.dma_start(out=wti.rearrange("p (f n) -> p f n", f=FIL),
                                  in_=wi_d.ap().rearrange("f (p n) -> p f n", p=N2))
                xts = []
                for b in range(BL):
                    xt = fp.tile([N2, N1 * C], F32R, tag=f"xt{b}")
                    nc.sync.dma_start(
                        out=xt, in_=xs_d.ap()[b].rearrange("(p q) c -> p (q c)", p=N2))
                    xts.append(xt)
                nc.sync.dma_start(out=ct["c_L"], in_=cd["c_L"].ap())

                Q = FIL // 4
                Hsbb = hp.tile([N2, 2 * FIL * N1], BF16, tag="Hsbb")
                BHc = hp.tile([N2, FIL * 2 * N1], BF16, tag="BHc")
                Asbs = []
                with tc.tile_pool(name="fxp", bufs=1, space="PSUM") as fps, \
                     tc.tile_pool(name="fhp", bufs=1, space="PSUM") as hps:
                    # --- PE: H-M1, then x-M1 (both b) ---
                    hps_t = hps.tile([N2, 2 * FIL * N1], F32, tag="Hps")
                    for c0, c1 in chunks_of(FIL * N1, MCH):
                        nc.tensor.matmul(hps_t[:, c0:c1], ct["c_F2r"], wtr[:, c0:c1],
                                         start=True, stop=False)
                        nc.tensor.matmul(hps_t[:, c0:c1], ct["c_F2in"], wti[:, c0:c1],
                                         start=False, stop=True)
                        d0 = FIL * N1
                        nc.tensor.matmul(hps_t[:, d0 + c0:d0 + c1], ct["c_F2i"],
                                         wtr[:, c0:c1], start=True, stop=False)
                        nc.tensor.matmul(hps_t[:, d0 + c0:d0 + c1], ct["c_F2r"],
                                         wti[:, c0:c1], start=False, stop=True)
                    xps = []
                    for b in range(BL):
                        ps = fps.tile([N2, 2 * N1 * C], F32, tag=f"Aps{b}")
                        for comp, w in ((0, "c_F2r"), (1, "c_F2i")):
                            for c0, c1 in chunks_of(N1 * C, MCH):
                                nc.tensor.matmul(
                                    ps[:, comp * N1 * C + c0: comp * N1 * C + c1],
                                    ct[w], xts[b][:, c0:c1], start=True, stop=True)
                        xps.append(ps)
                    # --- ACT: stage PSUM -> SBUF bf16 (layout (m, c, n) for x) ---
                    for b in range(BL):
                        Asb = fp.tile([N2, 2 * N1 * C], BF16, tag=f"Asb{b}")
                        Asbs.append(Asb)
                        nc.scalar.copy(
                            out=Asb.rearrange("p (m c n) -> p m n c", m=2, c=C),
                            in_=xps[b].rearrange("p (m n c) -> p m n c", m=2, c=C))
                    for q in (0, 2, 1, 3):
                        for d0 in (0, FIL * N1):
                            nc.scalar.copy(
                                out=Hsbb[:, d0 + q * Q * N1: d0 + (q + 1) * Q * N1],
                                in_=hps_t[:, d0 + q * Q * N1: d0 + (q + 1) * Q * N1])
                # PSUM released; all twiddles run from SBUF bf16 on DVE
                # --- x twiddle (DVE, bf16 2x): free order (c, n1) ---
                Bcs = []
                for b in range(BL):
                    Bc = fp.tile([N2, 2 * N1 * C], BF16, tag=f"Bc{b}")
                    Bcs.append(Bc)
                    u = fp.tile([N2, N1 * C], BF16, tag="u")
                    v = fp.tile([N2, N1 * C], BF16, tag="v")
                    Asv = Asbs[b].rearrange("p (m c n) -> p m c n", m=2, c=C)

                    def bcx(w):
                        return ct[w][:, None, :].broadcast_to([N2, C, N1])

                    uv = u.rearrange("p (c n) -> p c n", c=C)
                    vv = v.rearrange("p (c n) -> p c n", c=C)
                    Bv = Bc.rearrange("p (c m n) -> p m c n", c=C, m=2)
                    nc.vector.tensor_tensor(out=uv, in0=Asv[:, 0], in1=bcx("c_Twr"),
                                            op=AL.mult)
                    nc.vector.tensor_tensor(out=vv, in0=Asv[:, 1], in1=bcx("c_Twin"),
                                            op=AL.mult)
                    nc.vector.tensor_tensor(out=Bv[:, 0], in0=uv, in1=vv, op=AL.add)
                    nc.vector.tensor_tensor(out=uv, in0=Asv[:, 0], in1=bcx("c_Twi"),
                                            op=AL.mult)
                    nc.vector.tensor_tensor(out=vv, in0=Asv[:, 1], in1=bcx("c_Twr"),
                                            op=AL.mult)
                    nc.vector.tensor_tensor(out=Bv[:, 1], in0=uv, in1=vv, op=AL.add)

                def qtw(q):
                    # H twiddle for quarter q (DVE, bf16 2x); free order (f, n)
                    fsl = slice(q * Q * N1, (q + 1) * Q * N1)
                    Asrq = Hsbb[:, :FIL * N1][:, fsl].rearrange(
                        "p (f n) -> p f n", f=Q)
                    Asiq = Hsbb[:, FIL * N1:][:, fsl].rearrange(
                        "p (f n) -> p f n", f=Q)

                    def bchq(w):
                        return ct[w][:, None, :].broadcast_to([N2, Q, N1])

                    uhq = hp.tile([N2, Q * N1], BF16, tag="uh")
                    vhq = hp.tile([N2, Q * N1], BF16, tag="vh")
                    uvq = uhq.rearrange("p (f n) -> p f n", f=Q)
                    vvq = vhq.rearrange("p (f n) -> p f n", f=Q)
                    BHq = BHc[:, 2 * q * Q * N1:2 * (q + 1) * Q * N1].rearrange(
                        "p (f m n) -> p f m n", f=Q, m=2)
                    nc.vector.tensor_tensor(out=uvq, in0=Asrq, in1=bchq("c_Twr"),
                                            op=AL.mult)
                    nc.vector.tensor_tensor(out=vvq, in0=Asiq, in1=bchq("c_Twin"),
                                            op=AL.mult)
                    nc.vector.tensor_tensor(out=BHq[:, :, 0, :], in0=uvq, in1=vvq,
                                            op=AL.add)
                    nc.vector.tensor_tensor(out=uvq, in0=Asrq, in1=bchq("c_Twi"),
                                            op=AL.mult)
                    nc.vector.tensor_tensor(out=vvq, in0=Asiq, in1=bchq("c_Twr"),
                                            op=AL.mult)
                    nc.vector.tensor_tensor(out=BHq[:, :, 1, :], in0=uvq, in1=vvq,
                                            op=AL.add)

                Hs = hp.tile([N1s, KF], F32, tag="Hs")

                def qT(q, t1hps, m2hps):
                    # T1H transposes + M2H + Hs evac for quarter q
                    tp = t1hps.tile([N1s, 4 * N2], BF16, tag="t1h")
                    for j in range(4):
                        f = 4 * q + j
                        nc.tensor.transpose(
                            tp[:, j * N2:(j + 1) * N2],
                            BHc[:, f * 2 * N1:(f + 1) * 2 * N1], ct["c_idb"])
                    BTHq = hp.tile([N1s, 4 * N2], BF16, tag="BTH")
                    nc.scalar.copy(out=BTHq, in_=tp)
                    psq = m2hps.tile([N1s, 4 * N2], F32, tag="m2h")
                    nc.tensor.matmul(psq, ct["c_M2"], BTHq, start=True, stop=True)
                    nc.scalar.copy(out=Hs[:, q * KQ:(q + 1) * KQ], in_=psq)

                def ghalf(h):
                    qa, qb = QGROUPS[h]
                    HrP = gp.tile([N2, KQ], F32, tag="HrP")
                    HiP = gp.tile([N2, KQ], F32, tag="HiP")
                    nc.sync.dma_start(out=HrP[:N1, :],
                                      in_=Hs[:N1, qa * KQ:(qa + 1) * KQ])
                    nc.sync.dma_start(out=HrP[N1:, :],
                                      in_=Hs[:N1, qb * KQ:(qb + 1) * KQ])
                    nc.sync.dma_start(out=HiP[:N1, :],
                                      in_=Hs[N1:, qa * KQ:(qa + 1) * KQ])
                    nc.sync.dma_start(out=HiP[N1:, :],
                                      in_=Hs[N1:, qb * KQ:(qb + 1) * KQ])
                    sq1 = gp.tile([N2, KQ], F32, tag="sq1")
                    sq2 = gp.tile([N2, KQ], F32, tag="sq2")
                    nc.gpsimd.tensor_tensor(out=sq1, in0=HrP, in1=HrP, op=AL.mult)
                    nc.gpsimd.tensor_tensor(out=sq2, in0=HiP, in1=HiP, op=AL.mult)
                    nc.gpsimd.tensor_tensor(out=sq2, in0=sq1, in1=sq2, op=AL.add)
                    srp = srepP[:, h * KQ:(h + 1) * KQ]
                    nc.vector.tensor_tensor(out=sq2, in0=sq2, in1=srp, op=AL.add)
                    r = sq1
                    nc.vector.reciprocal(out=r, in_=sq2)
                    GrPb = gp.tile([N2, KQ], BF16, tag="GrPb")
                    GiPb = gp.tile([N2, KQ], BF16, tag="GiPb")
                    nc.vector.tensor_tensor(out=GrPb, in0=HrP, in1=r, op=AL.mult)
                    nc.vector.tensor_tensor(out=GiPb, in0=HiP, in1=r, op=AL.mult)
                    # unpack to stacked [Gr;Gr] / [gi;gi] with gi = Hi*r
                    # (the Im-G sign lives in c_M3p)
                    for (src, dstt) in ((GrPb, G1), (GiPb, G3)):
                        nc.sync.dma_start(out=dstt[:N1, qa * KQ:(qa + 1) * KQ],
                                          in_=src[:N1, :])
                        nc.sync.dma_start(out=dstt[:N1, qb * KQ:(qb + 1) * KQ],
                                          in_=src[N1:, :])
                        nc.sync.dma_start(out=dstt[N1:, qa * KQ:(qa + 1) * KQ],
                                          in_=src[:N1, :])
                        nc.sync.dma_start(out=dstt[N1:, qb * KQ:(qb + 1) * KQ],
                                          in_=src[N1:, :])

                qtw(0)
                qtw(2)
                with tc.tile_pool(name="m2hp", bufs=2, space="PSUM") as m2hps, \
                     tc.tile_pool(name="t1hp", bufs=2, space="PSUM") as t1hps:
                    qT(0, t1hps, m2hps)
                    qT(2, t1hps, m2hps)
                    ghalf(0)
                    qtw(1)
                    qtw(3)
                    qT(1, t1hps, m2hps)
                    qT(3, t1hps, m2hps)
                # --- x T1 transposes (4 per bank) + evac, M2, Z0A ---
                with tc.tile_pool(name="t1p", bufs=2, space="PSUM") as t1ps, \
                     tc.tile_pool(name="m2p", bufs=1, space="PSUM") as m2ps:
                    for b in range(BL):
                        Bview = Bcs[b].rearrange("p (c mn) -> p c mn", c=C)
                        for qq in range(C // 4):
                            tp = t1ps.tile([N1s, 4 * N2], BF16, tag="t1")
                            for j in range(4):
                                c = 4 * qq + j
                                nc.tensor.transpose(tp[:, j * N2:(j + 1) * N2],
                                                    Bview[:, c, :], ct["c_idb"])
                            row = b * C + 4 * qq
                            nc.scalar.copy(out=BT[:, row * N2:(row + 4) * N2], in_=tp)
                    psx = m2ps.tile([N1s, RN], F32, tag="m2")
                    for c0, c1 in chunks_of(RN, MCH):
                        nc.tensor.matmul(psx[:, c0:c1], ct["c_M2"], BT[:, c0:c1],
                                         start=True, stop=True)
                    nc.vector.tensor_copy(out=Z0A, in_=psx)
                ghalf(1)

            if debug_dumps:
                nc.gpsimd.dma_start(out=dbg["dZ0A"].ap(), in_=Z0A)
                nc.gpsimd.dma_start(out=dbg["dG1"].ap(), in_=G1)
                nc.gpsimd.dma_start(out=dbg["dG3"].ap(), in_=G3)
                nc.sync.dma_start(out=dbg["dHs"].ap(), in_=Hs)


            # ================= inverse units + M4, b-major =================
            DT = spool.tile([N2, BL * 2 * N1 * FIL * C], BF16, tag="DT")
            dtv = DT.rearrange("p (b ri n1 f c) -> p b ri n1 f c",
                               b=BL, ri=2, n1=N1, f=FIL)
            dt4 = DT.rearrange("p (b ri n1 fc) -> p b ri n1 fc", b=BL, ri=2, n1=N1)
            bkv = bk0.rearrange("p (f b c) -> p f b c", f=FIL, b=BL)
            NB = 8   # n1' per M4 PSUM group (2 banks)
            with tc.tile_pool(name="zt", bufs=3) as ztp, \
                 tc.tile_pool(name="invp", bufs=2, space="PSUM") as ips, \
                 tc.tile_pool(name="yp", bufs=2, space="PSUM") as yps, \
                 tc.tile_pool(name="yev", bufs=3) as yp:
                for b in range(BL):
                    for fi, f in enumerate(FORDER):
                        zb = Z0A[:, b * KB:(b + 1) * KB].rearrange(
                            "p (c k) -> p c k", c=C)
                        g1 = G1[:, f * N2:(f + 1) * N2][:, None, :].broadcast_to(
                            [N1s, C, N2])
                        g3 = G3[:, f * N2:(f + 1) * N2][:, None, :].broadcast_to(
                            [N1s, C, N2])
                        zt1 = ztp.tile([N1s, KB], BF16, tag="zt1")
                        zt3 = ztp.tile([N1s, KB], BF16, tag="zt3")
                        z1v = zt1.rearrange("p (c k) -> p c k", c=C)
                        z3v = zt3.rearrange("p (c k) -> p c k", c=C)
                        nc.vector.tensor_tensor(out=z1v, in0=zb, in1=g1, op=AL.mult)
                        meng = nc.gpsimd if fi % 2 == 1 else nc.vector
                        meng.tensor_tensor(out=z3v, in0=zb, in1=g3, op=AL.mult)
                        # bias into the k=0 bin (k1=0 real, k2=0) of zt1
                        z1k0 = zt1.rearrange("p (c k) -> p c k", c=C)[0:1, :, 0]
                        nc.vector.tensor_tensor(out=z1k0, in0=z1k0,
                                                in1=bkv[0:1, f, b], op=AL.add)
                        if debug_dumps and f == 0 and b == 0:
                            nc.sync.dma_start(out=dbg["dZT1"].ap(), in_=zt1)
                            nc.sync.dma_start(out=dbg["dZT3"].ap(), in_=zt3)
                        cps = ips.tile([N2, C * N1s], F32, tag="cps")
                        for c in range(C):
                            sl = cps[:, c * N1s:(c + 1) * N1s]
                            nc.tensor.matmul(sl, zt1[:, c * N2:(c + 1) * N2],
                                             ct["c_M3"], start=True, stop=False)
                            nc.tensor.matmul(sl, zt3[:, c * N2:(c + 1) * N2],
                                             ct["c_M3p"], start=False, stop=True)
                        cpv = cps.rearrange("p (c ri n1) -> p ri n1 c", c=C, ri=2)
                        if fi in (5, 11):
                            nc.vector.tensor_copy(out=dtv[:, b, :, :, f, :], in_=cpv)
                        else:
                            nc.scalar.copy(out=dtv[:, b, :, :, f, :], in_=cpv)
                    if debug_dumps and b == 0:
                        nc.gpsimd.dma_start(out=dbg["dDT"].ap(), in_=DT)
                    # ---- M4 for this batch (overlaps next batch's units) ----
                    for g0 in range(0, N1, NB):
                        ypsum = yps.tile([N2, NB * FC], F32, tag="yps")
                        for j in range(NB):
                            n1p = g0 + j
                            lr = ct["c_L"][:, n1p * N2:(n1p + 1) * N2]
                            li = ct["c_L"][:, (N1 + n1p) * N2:(N1 + n1p + 1) * N2]
                            sl = ypsum[:, j * FC:(j + 1) * FC]
                            nc.tensor.matmul(sl, lr, dt4[:, b, 0, n1p, :],
                                             start=True, stop=False)
                            nc.tensor.matmul(sl, li, dt4[:, b, 1, n1p, :],
                                             start=False, stop=True)
                        yt = yp.tile([N2, NB * FC], F32, tag="yt")
                        if b == 1 and (g0 // NB) % 2 == 1:
                            nc.vector.tensor_copy(out=yt, in_=ypsum)
                        else:
                            nc.scalar.copy(out=yt, in_=ypsum)
                        nc.sync.dma_start(
                            out=out_d.ap()[b].rearrange(
                                "(n2 n1) fc -> n2 n1 fc", n1=N1)[:, g0:g0 + NB, :],
                            in_=yt.rearrange("p (j fc) -> p j fc", j=NB))

    nc.compile()
    return nc


def chunks_of(total, step):
    return [(c0, min(total, c0 + step)) for c0 in range(0, total, step)]


def host_inputs(cfg, x_sh, w_real, w_imag, s, b):
    """Build the per-core in_map (numpy) for one core's batch shard."""
    import ml_dtypes
    cs = host_consts(cfg)
    N1, N2, T, FIL, C, BL = cfg.N1, cfg.N2, cfg.T, cfg.FIL, cfg.C, cfg.BL
    KQ = FIL * N2 // 4
    f32 = np.float32
    # packed s matching QGROUPS: half h rows 0..63 = quarter 2h? see QGROUPS
    S = np.broadcast_to(np.asarray(s, f32).reshape(FIL, 1), (FIL, N2)).reshape(-1)
    halves = []
    for (qa, qb) in QGROUPS:
        halves.append(np.concatenate([
            np.broadcast_to(S[qa * KQ:(qa + 1) * KQ], (N1, KQ)),
            np.broadcast_to(S[qb * KQ:(qb + 1) * KQ], (N1, KQ))], axis=0))
    srepP = np.concatenate(halves, axis=1).astype(f32).copy()
    bf = np.asarray(b, f32).reshape(FIL, C)
    bk0 = np.broadcast_to((T * bf)[:, None, :], (FIL, BL, C)).reshape(1, -1)
    m = {
        "xs": np.ascontiguousarray(x_sh, dtype=f32),
        "wr": np.ascontiguousarray(w_real, dtype=f32),
        "wi": np.ascontiguousarray(w_imag, dtype=f32),
        "srepP": srepP,
        "bk0": bk0.astype(f32).copy(),
    }
    for k, v in cs.items():
        if k in ("c_L", "c_M2", "c_M3", "c_M3p", "c_idb", "c_Twr", "c_Twi",
                 "c_Twin"):
            m[k] = v.astype(ml_dtypes.bfloat16)
        else:
            m[k] = v
    return m


_NC_CACHE = {}


def kernel(x, w_real, w_imag, s, b):
    """Full-input entry point: shard over 8 cores, run, gather."""
    from concourse.bass_utils import run_bass_kernel_spmd
    cfg = FULL
    n_cores = 8
    key = "full"
    if key not in _NC_CACHE:
        _NC_CACHE[key] = build_nc(cfg)
    nc = _NC_CACHE[key]
    x = np.asarray(x, dtype=np.float32)
    w_real = np.asarray(w_real, dtype=np.float32)
    w_imag = np.asarray(w_imag, dtype=np.float32)
    s = np.asarray(s, dtype=np.float32)
    b = np.asarray(b, dtype=np.float32)
    in_maps = []
    for i in range(n_cores):
        x_sh = x[i * cfg.BL:(i + 1) * cfg.BL]
        in_maps.append(host_inputs(cfg, x_sh, w_real, w_imag, s, b))
    res = run_bass_kernel_spmd(nc, in_maps, core_ids=list(range(n_cores)))
    outs = [res.results[i]["out"] for i in range(n_cores)]
    return np.concatenate(outs, axis=0).astype(np.float32)


# revision 35
# speedup vs baseline: 1.5186x; 1.0246x over previous
"""Trainium2 Bass kernel: frequency-domain regularized (Wiener) deconvolution.

Reference computation (B=16, T=8192, C=8, FIL=16):
    h  = fft(w_real + i*w_imag)            # (FIL, T)
    g  = conj(h) / (|h|^2 + s)             # (FIL, T)
    xf = fft(x, axis=T)                    # per (b, c) row
    y  = real(ifft(xf[:,None,:,:] * g[None,:,None,:]))
    out = y -> (B, T, FIL*C) + bias

Sharding: data-parallel over batch across 8 cores (2 batches/core); filter
params replicated.  FFTs are 4-step Cooley-Tukey matmuls on the PE array
(T = N2*N1, N2=128, N1=64; n = n1 + N1*n2, k = k2 + N2*k1):

  forward:  M1 (contract n2, fp32r) -> twiddle W^(n1 k2) (DVE real half,
            Pool imag half) -> PE transpose T1 (4 per PSUM bank, ACT evac)
            -> M2 (contract n1, stacked-complex K) -> Z0A [k1r;k1i|(b,c,k2)]
  filter:   H-path processed in f-QUARTERS pipelined through (ACT stage,
            Pool/DVE twiddle, T1H, M2H, Hs evac); G pipeline runs twice on
            partition-packed [128, 512] halves covering f {0-3,8-11} then
            {4-7,12-15}; assembled into stacked bf16 G1=[Gr;Gr], G3=[Gi;Gi]
  inverse:  per (b,f) unit: zt1=Z0A_b*G1f, zt3=Z0A_b*G3f on DVE (some zt3
            on Pool); bias folded into the k=0 bin of zt1 (tiny DVE op);
            stage-1 iFFT contracts k1 with the DATA as matmul weights
            (out partitions = k2) and the complex add fused into PSUM
            accumulation via two weight matrices c_M3/c_M3p -> no PE
            transpose, no DVE add, no swapped Z0B copy; ACT evacuates
            [k2 | (ri,n1',f,c)] bf16 per unit; M4 contracts k2 per (b,n1')
            with inverse twiddle folded into static bf16 weights.  Units run
            b-major so M4(b0) overlaps batch-1's unit pipeline.
"""
import sys

sys.path.insert(0, "/opt/trn_rl_repo")

import numpy as np


def _get_cc():
    import concourse.bacc as bacc
    import concourse.mybir as mybir
    import concourse.tile as tile
    return bacc, mybir, tile


class Cfg:
    def __init__(self, T=8192, N2=128, N1=64, BL=2, C=8, FIL=16):
        assert N1 * N2 == T
        self.T, self.N2, self.N1, self.BL, self.C, self.FIL = T, N2, N1, BL, C, FIL
        self.ROWS = BL * C
        self.FC = FIL * C


FULL = Cfg()


def host_consts(cfg):
    """Static (input-independent) weights, as fp32 numpy arrays."""
    T, N1, N2 = cfg.T, cfg.N1, cfg.N2
    f32 = np.float32
    cs = {}
    a2 = np.arange(N2)
    a1 = np.arange(N1)
    F2 = np.exp(-2j * np.pi * np.outer(a2, a2) / N2)        # [n2, k2]
    cs["c_F2r"] = F2.real.astype(f32)
    cs["c_F2i"] = F2.imag.astype(f32)
    cs["c_F2in"] = (-F2.imag).astype(f32)
    Tw = np.exp(-2j * np.pi * np.outer(a2, a1) / T)         # [k2, n1]
    cs["c_Twr"] = Tw.real.astype(f32)
    cs["c_Twi"] = Tw.imag.astype(f32)
    cs["c_Twin"] = (-Tw.imag).astype(f32)
    F1 = np.exp(-2j * np.pi * np.outer(a1, a1) / N1)        # [n1, k1]
    cs["c_M2"] = np.hstack([np.vstack([F1.real, -F1.imag]),
                            np.vstack([F1.imag, F1.real])]).astype(f32)
    Fb1 = np.exp(2j * np.pi * np.outer(a1, a1) / N1)        # [k1, n1']
    M3 = np.hstack([np.vstack([Fb1.real, -Fb1.imag]),
                    np.vstack([Fb1.imag, Fb1.real])]).astype(f32)
    cs["c_M3"] = M3
    # row-swapped/sign-flipped variant: with zt3 = [Zr*gi; Zi*gi] where
    # gi = Hi*r = -Im(G),  zt3^T @ c_M3p == zt2^T @ c_M3 for the old
    # zt2 = [Zi*gi; -Zr*gi] (stacked-swap complex-multiply half)
    cs["c_M3p"] = np.vstack([-M3[N1:], M3[:N1]]).astype(f32)
    # M4 per-n1' weights, inverse twiddle folded in:
    #   L_{n1'}[k2, n2'] = exp(+2j pi k2 n2'/N2) * exp(+2j pi n1' k2 / T) / T
    Fb2 = np.exp(2j * np.pi * np.outer(a2, a2) / N2)        # [k2, n2']
    ph = np.exp(2j * np.pi * np.outer(a1, a2) / T)          # [n1', k2]
    L = Fb2[None, :, :] * ph[:, :, None] / T                # [n1', k2, n2']
    Lr = L.real.transpose(1, 0, 2).reshape(N2, N1 * N2)     # [k2, (n1', n2')]
    Lin = (-L.imag).transpose(1, 0, 2).reshape(N2, N1 * N2)
    cs["c_L"] = np.concatenate([Lr, Lin], axis=1).astype(f32)  # [k2 | (ri, n1', n2')]
    cs["c_idb"] = np.eye(N2, dtype=f32)
    return cs


# f-quarters: packed-G half 0 covers quarters (0, 2) = f {0..3, 8..11}
QGROUPS = [(0, 2), (1, 3)]
FORDER = [0, 1, 2, 3, 8, 9, 10, 11, 4, 5, 6, 7, 12, 13, 14, 15]


def build_nc(cfg, debug_dumps=False):
    bacc, mybir, tile = _get_cc()
    F32, F32R, BF16 = mybir.dt.float32, mybir.dt.float32r, mybir.dt.bfloat16
    AL = mybir.AluOpType
    T, N1, N2, BL, C, FIL = cfg.T, cfg.N1, cfg.N2, cfg.BL, cfg.C, cfg.FIL
    ROWS, FC = cfg.ROWS, cfg.FC
    N1s = 2 * N1          # stacked (real; imag) partition dim = 128
    KF = FIL * N2         # H/G free size, (f, k2) order = 2048
    KH = KF // 2          # packed layout free size = 1024
    KQ = KF // 4          # one f-quarter = 512
    RN = ROWS * N2        # Z0 free size, (b, c, k2) order = 2048
    KB = C * N2           # per-(b,f) free size = 1024
    MCH = 512

    nc = bacc.Bacc("TRN2", debug=False)

    xs_d = nc.dram_tensor("xs", [BL, T, C], F32R, kind="ExternalInput")
    wr_d = nc.dram_tensor("wr", [FIL, T], F32R, kind="ExternalInput")
    wi_d = nc.dram_tensor("wi", [FIL, T], F32R, kind="ExternalInput")
    srepP_d = nc.dram_tensor("srepP", [N2, KH], F32, kind="ExternalInput")
    bk0_d = nc.dram_tensor("bk0", [1, FIL * BL * C], F32, kind="ExternalInput")
    cdef = [
        ("c_F2r", [N2, N2], F32R), ("c_F2i", [N2, N2], F32R), ("c_F2in", [N2, N2], F32R),
        ("c_Twr", [N2, N1], BF16), ("c_Twi", [N2, N1], BF16), ("c_Twin", [N2, N1], BF16),
        ("c_M2", [N1s, N1s], BF16), ("c_M3", [N1s, N1s], BF16),
        ("c_M3p", [N1s, N1s], BF16),
        ("c_L", [N2, 2 * N1 * N2], BF16),
        ("c_idb", [N2, N2], BF16),
    ]
    cd = {}
    for name, shape, dt_ in cdef:
        cd[name] = nc.dram_tensor(name, shape, dt_, kind="ExternalInput")
    out_d = nc.dram_tensor("out", [BL, T, FC], F32, kind="ExternalOutput")
    dbg = {}
    if debug_dumps:
        for nm, shape, ddt in [("dZ0A", [N1s, RN], BF16),
                               ("dG1", [N1s, KF], BF16), ("dG3", [N1s, KF], BF16),
                               ("dZT1", [N1s, KB], BF16), ("dZT3", [N1s, KB], BF16),
                               ("dHs", [N1s, KF], F32),
                               ("dDT", [N2, BL * 2 * N1 * FIL * C], BF16)]:
            dbg[nm] = nc.dram_tensor(nm, shape, ddt, kind="ExternalOutput")

    with tile.TileContext(nc) as tc:
        with tc.tile_pool(name="consts", bufs=1) as cpool, \
             tc.tile_pool(name="spec", bufs=1) as spool:
            ct = {}
            for name, shape, dt_ in cdef:
                t_ = cpool.tile(shape, dt_, tag=name)
                ct[name] = t_
            bk0 = cpool.tile([1, FIL * BL * C], F32, tag="bk0")
            srepP = cpool.tile([N2, KH], F32, tag="srepP")

            def load_consts(names):
                for name in names:
                    nc.sync.dma_start(out=ct[name], in_=cd[name].ap())

            Z0A = spool.tile([N1s, RN], BF16, tag="Z0A")   # [k1r;k1i | (b,c,k2)]
            G1 = spool.tile([N1s, KF], BF16, tag="G1")     # [Gr;Gr | (f,k2)]
            G3 = spool.tile([N1s, KF], BF16, tag="G3")     # [Gi;Gi | (f,k2)]
            BT = spool.tile([N1s, RN], BF16, tag="BT")     # [n1r;n1i | (b,c,k2)]

            # ============ H forward (f-quarters) + x forward, interleaved ===
            with tc.tile_pool(name="fh", bufs=1) as hp, \
                 tc.tile_pool(name="fx", bufs=1) as fp, \
                 tc.tile_pool(name="gp", bufs=1) as gp:
                xts = []
                for b in range(BL):
                    xt = fp.tile([N2, N1 * C], F32R, tag=f"xt{b}")
                    nc.sync.dma_start(
                        out=xt, in_=xs_d.ap()[b].rearrange("(p q) c -> p (q c)", p=N2))
                    xts.append(xt)
                load_consts(["c_F2r", "c_F2i", "c_F2in"])
                wtr = hp.tile([N2, FIL * N1], F32R, tag="wtr")
                wti = hp.tile([N2, FIL * N1], F32R, tag="wti")
                nc.sync.dma_start(out=wtr.rearrange("p (f n) -> p f n", f=FIL),
                                  in_=wr_d.ap().rearrange("f (p n) -> p f n", p=N2))
                nc.sync.dma_start(out=wti.rearrange("p (f n) -> p f n", f=FIL),
                                  in_=wi_d.ap().rearrange("f (p n) -> p f n", p=N2))
                load_consts(["c_Twr", "c_Twi", "c_Twin", "c_M2", "c_idb",
                             "c_M3", "c_M3p"])
                nc.sync.dma_start(out=bk0, in_=bk0_d.ap())
                nc.sync.dma_start(out=srepP, in_=srepP_d.ap())

                Q = FIL // 4
                Hsbb = hp.tile([N2, 2 * FIL * N1], BF16, tag="Hsbb")
                BHc = hp.tile([N2, FIL * 2 * N1], BF16, tag="BHc")
                Asbs = []
                with tc.tile_pool(name="fxp", bufs=1, space="PSUM") as fps, \
                     tc.tile_pool(name="fhp", bufs=1, space="PSUM") as hps:
                    # --- PE: x-M1 (both b) first, then H-M1 ---
                    xps = []
                    for b in range(BL):
                        ps = fps.tile([N2, 2 * N1 * C], F32, tag=f"Aps{b}")
                        for comp, w in ((0, "c_F2r"), (1, "c_F2i")):
                            for c0, c1 in chunks_of(N1 * C, MCH):
                                nc.tensor.matmul(
                                    ps[:, comp * N1 * C + c0: comp * N1 * C + c1],
                                    ct[w], xts[b][:, c0:c1], start=True, stop=True)
                        xps.append(ps)
                    hps_t = hps.tile([N2, 2 * FIL * N1], F32, tag="Hps")
                    for c0, c1 in chunks_of(FIL * N1, MCH):
                        nc.tensor.matmul(hps_t[:, c0:c1], ct["c_F2r"], wtr[:, c0:c1],
                                         start=True, stop=False)
                        nc.tensor.matmul(hps_t[:, c0:c1], ct["c_F2in"], wti[:, c0:c1],
                                         start=False, stop=True)
                        d0 = FIL * N1
                        nc.tensor.matmul(hps_t[:, d0 + c0:d0 + c1], ct["c_F2i"],
                                         wtr[:, c0:c1], start=True, stop=False)
                        nc.tensor.matmul(hps_t[:, d0 + c0:d0 + c1], ct["c_F2r"],
                                         wti[:, c0:c1], start=False, stop=True)
                    # --- ACT: stage PSUM -> SBUF bf16 (layout (m, c, n) for x) ---
                    for b in range(BL):
                        Asb = fp.tile([N2, 2 * N1 * C], BF16, tag=f"Asb{b}")
                        Asbs.append(Asb)
                        nc.scalar.copy(
                            out=Asb.rearrange("p (m c n) -> p m n c", m=2, c=C),
                            in_=xps[b].rearrange("p (m n c) -> p m n c", m=2, c=C))
                    for q in (0, 2, 1, 3):
                        for d0 in (0, FIL * N1):
                            nc.scalar.copy(
                                out=Hsbb[:, d0 + q * Q * N1: d0 + (q + 1) * Q * N1],
                                in_=hps_t[:, d0 + q * Q * N1: d0 + (q + 1) * Q * N1])
                # PSUM released; all twiddles run from SBUF bf16 on DVE
                # --- x twiddle (DVE, bf16 2x): free order (c, n1) ---
                Bcs = []
                for b in range(BL):
                    Bc = fp.tile([N2, 2 * N1 * C], BF16, tag=f"Bc{b}")
                    Bcs.append(Bc)
                    u = fp.tile([N2, N1 * C], BF16, tag="u")
                    v = fp.tile([N2, N1 * C], BF16, tag="v")
                    Asv = Asbs[b].rearrange("p (m c n) -> p m c n", m=2, c=C)

                    def bcx(w):
                        return ct[w][:, None, :].broadcast_to([N2, C, N1])

                    uv = u.rearrange("p (c n) -> p c n", c=C)
                    vv = v.rearrange("p (c n) -> p c n", c=C)
                    Bv = Bc.rearrange("p (c m n) -> p m c n", c=C, m=2)
                    nc.vector.tensor_tensor(out=uv, in0=Asv[:, 0], in1=bcx("c_Twr"),
                                            op=AL.mult)
                    nc.vector.tensor_tensor(out=vv, in0=Asv[:, 1], in1=bcx("c_Twin"),
                                            op=AL.mult)
                    nc.vector.tensor_tensor(out=Bv[:, 0], in0=uv, in1=vv, op=AL.add)
                    nc.vector.tensor_tensor(out=uv, in0=Asv[:, 0], in1=bcx("c_Twi"),
                                            op=AL.mult)
                    nc.vector.tensor_tensor(out=vv, in0=Asv[:, 1], in1=bcx("c_Twr"),
                                            op=AL.mult)
                    nc.vector.tensor_tensor(out=Bv[:, 1], in0=uv, in1=vv, op=AL.add)

                def qtw(q):
                    # H twiddle for quarter q (DVE, bf16 2x); free order (f, n)
                    fsl = slice(q * Q * N1, (q + 1) * Q * N1)
                    Asrq = Hsbb[:, :FIL * N1][:, fsl].rearrange(
                        "p (f n) -> p f n", f=Q)
                    Asiq = Hsbb[:, FIL * N1:][:, fsl].rearrange(
                        "p (f n) -> p f n", f=Q)

                    def bchq(w):
                        return ct[w][:, None, :].broadcast_to([N2, Q, N1])

                    uhq = hp.tile([N2, Q * N1], BF16, tag="uh")
                    vhq = hp.tile([N2, Q * N1], BF16, tag="vh")
                    uvq = uhq.rearrange("p (f n) -> p f n", f=Q)
                    vvq = vhq.rearrange("p (f n) -> p f n", f=Q)
                    BHq = BHc[:, 2 * q * Q * N1:2 * (q + 1) * Q * N1].rearrange(
                        "p (f m n) -> p f m n", f=Q, m=2)
                    nc.vector.tensor_tensor(out=uvq, in0=Asrq, in1=bchq("c_Twr"),
                                            op=AL.mult)
                    nc.vector.tensor_tensor(out=vvq, in0=Asiq, in1=bchq("c_Twin"),
                                            op=AL.mult)
                    nc.vector.tensor_tensor(out=BHq[:, :, 0, :], in0=uvq, in1=vvq,
                                            op=AL.add)
                    nc.vector.tensor_tensor(out=uvq, in0=Asrq, in1=bchq("c_Twi"),
                                            op=AL.mult)
                    nc.vector.tensor_tensor(out=vvq, in0=Asiq, in1=bchq("c_Twr"),
                                            op=AL.mult)
                    nc.vector.tensor_tensor(out=BHq[:, :, 1, :], in0=uvq, in1=vvq,
                                            op=AL.add)

                Hs = hp.tile([N1s, KF], F32, tag="Hs")

                def qT(q, t1hps, m2hps):
                    # T1H transposes + M2H + Hs evac for quarter q
                    tp = t1hps.tile([N1s, 4 * N2], BF16, tag="t1h")
                    for j in range(4):
                        f = 4 * q + j
                        nc.tensor.transpose(
                            tp[:, j * N2:(j + 1) * N2],
                            BHc[:, f * 2 * N1:(f + 1) * 2 * N1], ct["c_idb"])
                    BTHq = hp.tile([N1s, 4 * N2], BF16, tag="BTH")
                    nc.scalar.copy(out=BTHq, in_=tp)
                    psq = m2hps.tile([N1s, 4 * N2], F32, tag="m2h")
                    nc.tensor.matmul(psq, ct["c_M2"], BTHq, start=True, stop=True)
                    nc.scalar.copy(out=Hs[:, q * KQ:(q + 1) * KQ], in_=psq)

                def ghalf(h):
                    qa, qb = QGROUPS[h]
                    HrP = gp.tile([N2, KQ], F32, tag="HrP")
                    HiP = gp.tile([N2, KQ], F32, tag="HiP")
                    nc.sync.dma_start(out=HrP[:N1, :],
                                      in_=Hs[:N1, qa * KQ:(qa + 1) * KQ])
                    nc.sync.dma_start(out=HrP[N1:, :],
                                      in_=Hs[:N1, qb * KQ:(qb + 1) * KQ])
                    nc.sync.dma_start(out=HiP[:N1, :],
                                      in_=Hs[N1:, qa * KQ:(qa + 1) * KQ])
                    nc.sync.dma_start(out=HiP[N1:, :],
                                      in_=Hs[N1:, qb * KQ:(qb + 1) * KQ])
                    sq1 = gp.tile([N2, KQ], F32, tag="sq1")
                    sq2 = gp.tile([N2, KQ], F32, tag="sq2")
                    nc.gpsimd.tensor_tensor(out=sq1, in0=HrP, in1=HrP, op=AL.mult)
                    nc.gpsimd.tensor_tensor(out=sq2, in0=HiP, in1=HiP, op=AL.mult)
                    nc.gpsimd.tensor_tensor(out=sq2, in0=sq1, in1=sq2, op=AL.add)
                    srp = srepP[:, h * KQ:(h + 1) * KQ]
                    nc.vector.tensor_tensor(out=sq2, in0=sq2, in1=srp, op=AL.add)
                    r = sq1
                    nc.vector.reciprocal(out=r, in_=sq2)
                    GrPb = gp.tile([N2, KQ], BF16, tag="GrPb")
                    GiPb = gp.tile([N2, KQ], BF16, tag="GiPb")
                    nc.vector.tensor_tensor(out=GrPb, in0=HrP, in1=r, op=AL.mult)
                    nc.vector.tensor_tensor(out=GiPb, in0=HiP, in1=r, op=AL.mult)
                    # unpack to stacked [Gr;Gr] / [gi;gi] with gi = Hi*r
                    # (the Im-G sign lives in c_M3p)
                    for (src, dstt) in ((GrPb, G1), (GiPb, G3)):
                        nc.sync.dma_start(out=dstt[:N1, qa * KQ:(qa + 1) * KQ],
                                          in_=src[:N1, :])
                        nc.sync.dma_start(out=dstt[:N1, qb * KQ:(qb + 1) * KQ],
                                          in_=src[N1:, :])
                        nc.sync.dma_start(out=dstt[N1:, qa * KQ:(qa + 1) * KQ],
                                          in_=src[:N1, :])
                        nc.sync.dma_start(out=dstt[N1:, qb * KQ:(qb + 1) * KQ],
                                          in_=src[N1:, :])

                qtw(0)
                qtw(2)
                with tc.tile_pool(name="m2hp", bufs=2, space="PSUM") as m2hps, \
                     tc.tile_pool(name="t1hp", bufs=2, space="PSUM") as t1hps:
                    qT(0, t1hps, m2hps)
                    qT(2, t1hps, m2hps)
                    ghalf(0)
                    qtw(1)
                    qtw(3)
                    qT(1, t1hps, m2hps)
                    qT(3, t1hps, m2hps)
                # --- x T1 transposes (4 per bank) + evac, M2, Z0A ---
                with tc.tile_pool(name="t1p", bufs=2, space="PSUM") as t1ps, \
                     tc.tile_pool(name="m2p", bufs=1, space="PSUM") as m2ps:
                    for b in range(BL):
                        Bview = Bcs[b].rearrange("p (c mn) -> p c mn", c=C)
                        for qq in range(C // 4):
                            tp = t1ps.tile([N1s, 4 * N2], BF16, tag="t1")
                            for j in range(4):
                                c = 4 * qq + j
                                nc.tensor.transpose(tp[:, j * N2:(j + 1) * N2],
                                                    Bview[:, c, :], ct["c_idb"])
                            row = b * C + 4 * qq
                            nc.scalar.copy(out=BT[:, row * N2:(row + 4) * N2], in_=tp)
                    psx = m2ps.tile([N1s, RN], F32, tag="m2")
                    for c0, c1 in chunks_of(RN, MCH):
                        nc.tensor.matmul(psx[:, c0:c1], ct["c_M2"], BT[:, c0:c1],
                                         start=True, stop=True)
                    nc.vector.tensor_copy(out=Z0A, in_=psx)
                ghalf(1)
                nc.sync.dma_start(out=ct["c_L"], in_=cd["c_L"].ap())

            if debug_dumps:
                nc.gpsimd.dma_start(out=dbg["dZ0A"].ap(), in_=Z0A)
                nc.gpsimd.dma_start(out=dbg["dG1"].ap(), in_=G1)
                nc.gpsimd.dma_start(out=dbg["dG3"].ap(), in_=G3)
                nc.sync.dma_start(out=dbg["dHs"].ap(), in_=Hs)


            # ================= inverse units + M4, b-major =================
            DT = spool.tile([N2, BL * 2 * N1 * FIL * C], BF16, tag="DT")
            dtv = DT.rearrange("p (b ri n1 f c) -> p b ri n1 f c",
                               b=BL, ri=2, n1=N1, f=FIL)
            dt4 = DT.rearrange("p (b ri n1 fc) -> p b ri n1 fc", b=BL, ri=2, n1=N1)
            bkv = bk0.rearrange("p (f b c) -> p f b c", f=FIL, b=BL)
            NB = 4   # n1' per M4 PSUM group (1 bank)
            with tc.tile_pool(name="zt", bufs=3) as ztp, \
                 tc.tile_pool(name="invp", bufs=3, space="PSUM") as ips, \
                 tc.tile_pool(name="yp", bufs=2, space="PSUM") as yps, \
                 tc.tile_pool(name="yev", bufs=3) as yp:
                for b in range(BL):
                    for fi, f in enumerate(FORDER):
                        zb = Z0A[:, b * KB:(b + 1) * KB].rearrange(
                            "p (c k) -> p c k", c=C)
                        g1 = G1[:, f * N2:(f + 1) * N2][:, None, :].broadcast_to(
                            [N1s, C, N2])
                        g3 = G3[:, f * N2:(f + 1) * N2][:, None, :].broadcast_to(
                            [N1s, C, N2])
                        zt1 = ztp.tile([N1s, KB], BF16, tag="zt1")
                        zt3 = ztp.tile([N1s, KB], BF16, tag="zt3")
                        z1v = zt1.rearrange("p (c k) -> p c k", c=C)
                        z3v = zt3.rearrange("p (c k) -> p c k", c=C)
                        nc.vector.tensor_tensor(out=z1v, in0=zb, in1=g1, op=AL.mult)
                        meng = nc.gpsimd if fi % 2 == 1 else nc.vector
                        meng.tensor_tensor(out=z3v, in0=zb, in1=g3, op=AL.mult)
                        # bias into the k=0 bin (k1=0 real, k2=0) of zt1
                        z1k0 = zt1.rearrange("p (c k) -> p c k", c=C)[0:1, :, 0]
                        nc.vector.tensor_tensor(out=z1k0, in0=z1k0,
                                                in1=bkv[0:1, f, b], op=AL.add)
                        if debug_dumps and f == 0 and b == 0:
                            nc.sync.dma_start(out=dbg["dZT1"].ap(), in_=zt1)
                            nc.sync.dma_start(out=dbg["dZT3"].ap(), in_=zt3)
                        cps = ips.tile([N2, C * N1s], F32, tag="cps")
                        for c in range(C):
                            sl = cps[:, c * N1s:(c + 1) * N1s]
                            nc.tensor.matmul(sl, zt1[:, c * N2:(c + 1) * N2],
                                             ct["c_M3"], start=True, stop=False)
                            nc.tensor.matmul(sl, zt3[:, c * N2:(c + 1) * N2],
                                             ct["c_M3p"], start=False, stop=True)
                        cpv = cps.rearrange("p (c ri n1) -> p ri n1 c", c=C, ri=2)
                        if fi in (5, 11):
                            nc.vector.tensor_copy(out=dtv[:, b, :, :, f, :], in_=cpv)
                        else:
                            nc.scalar.copy(out=dtv[:, b, :, :, f, :], in_=cpv)
                    if debug_dumps and b == 0:
                        nc.gpsimd.dma_start(out=dbg["dDT"].ap(), in_=DT)
                    # ---- M4 for this batch (overlaps next batch's units) ----
                    for g0 in range(0, N1, NB):
                        ypsum = yps.tile([N2, NB * FC], F32, tag="yps")
                        for j in range(NB):
                            n1p = g0 + j
                            lr = ct["c_L"][:, n1p * N2:(n1p + 1) * N2]
                            li = ct["c_L"][:, (N1 + n1p) * N2:(N1 + n1p + 1) * N2]
                            sl = ypsum[:, j * FC:(j + 1) * FC]
                            nc.tensor.matmul(sl, lr, dt4[:, b, 0, n1p, :],
                                             start=True, stop=False)
                            nc.tensor.matmul(sl, li, dt4[:, b, 1, n1p, :],
                                             start=False, stop=True)
                        yt = yp.tile([N2, NB * FC], F32, tag="yt")
                        if b == 1 and (g0 // NB) % 2 == 1:
                            nc.vector.tensor_copy(out=yt, in_=ypsum)
                        else:
                            nc.scalar.copy(out=yt, in_=ypsum)
                        nc.sync.dma_start(
                            out=out_d.ap()[b].rearrange(
                                "(n2 n1) fc -> n2 n1 fc", n1=N1)[:, g0:g0 + NB, :],
                            in_=yt.rearrange("p (j fc) -> p j fc", j=NB))

    nc.compile()
    return nc


def chunks_of(total, step):
    return [(c0, min(total, c0 + step)) for c0 in range(0, total, step)]


def host_inputs(cfg, x_sh, w_real, w_imag, s, b):
    """Build the per-core in_map (numpy) for one core's batch shard."""
    import ml_dtypes
    cs = host_consts(cfg)
    N1, N2, T, FIL, C, BL = cfg.N1, cfg.N2, cfg.T, cfg.FIL, cfg.C, cfg.BL
    KQ = FIL * N2 // 4
    f32 = np.float32
    # packed s matching QGROUPS: half h rows 0..63 = quarter 2h? see QGROUPS
    S = np.broadcast_to(np.asarray(s, f32).reshape(FIL, 1), (FIL, N2)).reshape(-1)
    halves = []
    for (qa, qb) in QGROUPS:
        halves.append(np.concatenate([
            np.broadcast_to(S[qa * KQ:(qa + 1) * KQ], (N1, KQ)),
            np.broadcast_to(S[qb * KQ:(qb + 1) * KQ], (N1, KQ))], axis=0))
    srepP = np.concatenate(halves, axis=1).astype(f32).copy()
    bf = np.asarray(b, f32).reshape(FIL, C)
    bk0 = np.broadcast_to((T * bf)[:, None, :], (FIL, BL, C)).reshape(1, -1)
    m = {
        "xs": np.ascontiguousarray(x_sh, dtype=f32),
        "wr": np.ascontiguousarray(w_real, dtype=f32),
        "wi": np.ascontiguousarray(w_imag, dtype=f32),
        "srepP": srepP,
        "bk0": bk0.astype(f32).copy(),
    }
    for k, v in cs.items():
        if k in ("c_L", "c_M2", "c_M3", "c_M3p", "c_idb", "c_Twr", "c_Twi",
                 "c_Twin"):
            m[k] = v.astype(ml_dtypes.bfloat16)
        else:
            m[k] = v
    return m


_NC_CACHE = {}


def kernel(x, w_real, w_imag, s, b):
    """Full-input entry point: shard over 8 cores, run, gather."""
    from concourse.bass_utils import run_bass_kernel_spmd
    cfg = FULL
    n_cores = 8
    key = "full"
    if key not in _NC_CACHE:
        _NC_CACHE[key] = build_nc(cfg)
    nc = _NC_CACHE[key]
    x = np.asarray(x, dtype=np.float32)
    w_real = np.asarray(w_real, dtype=np.float32)
    w_imag = np.asarray(w_imag, dtype=np.float32)
    s = np.asarray(s, dtype=np.float32)
    b = np.asarray(b, dtype=np.float32)
    in_maps = []
    for i in range(n_cores):
        x_sh = x[i * cfg.BL:(i + 1) * cfg.BL]
        in_maps.append(host_inputs(cfg, x_sh, w_real, w_imag, s, b))
    res = run_bass_kernel_spmd(nc, in_maps, core_ids=list(range(n_cores)))
    outs = [res.results[i]["out"] for i in range(n_cores)]
    return np.concatenate(outs, axis=0).astype(np.float32)
